# revision 3
# baseline (speedup 1.0000x reference)
"""Distance-estimator (2x RGCN encoder + regressor) on 8 TRN2 NeuronCores.

Sharding: cores 0-3 = state encoder, 4-7 = goal encoder (SPMD, different
data). Each core owns a contiguous quarter of the graphs -> contiguous
dst-node range (batch sorted). Per core-layer:

Phase 1: edges sorted by (src-chunk, type[, parity]); dma_gather source rows
  (bf16, transposed) from the feature table; per-128-token matmul with W_type
  -> msgs [tok, 64] f32 written sequentially to an HBM buffer.
Phase 2: msgs re-gathered in (dst-range, msgs-slice) order: for each range
  of 2048 dst nodes x each 32k-row msgs slice (int16 idx), gather the
  tokens, scale by 1/cnt(dst,type), aggregate with indicator matmuls
  (lhsT = one-hot of dst-within-group built via iota/is_equal) writing
  dense node-columns of a PSUM [64, 2048] accumulator -> acc [64h, N_PAD]
  bf16 image (transposed orientation).
Epilogue (transposed): accT + Wroot^T@xT + b -> relu -> hT tiles; L1 emits
  the pair-packed bf16 h table (AllGather across the encoder's 4 cores);
  L2 pools via matmul with host P matrices; paired cores exchange pooled
  embeddings; per-core f32 regressor for its 128 graphs.
"""
import os
import sys
sys.path.insert(0, "/opt/trn_rl_repo")
STAGE = int(os.environ.get("K_STAGE", "99"))
SUB = int(os.environ.get("K_SUB", "15"))
MAXCALL = int(os.environ.get("K_MAXCALL", "100000"))
import numpy as np
import ml_dtypes

from concourse import bass, bacc, tile
from concourse import bass_utils
from concourse.bass import mybir
from concourse.alu_op_type import AluOpType

DT = mybir.dt
BF16 = ml_dtypes.bfloat16
LAST_DBG = None
LAST_PLANS = None
LAST_RES = None

IN = 128
H = 64
R = 32
TOK = 128
CALL_G = int(os.environ.get("K_CALLG", "16"))
GMAX = 2048              # max idxs per dma_gather call
RANGE_D = 2048           # dsts per phase-2 range (psum [64, 2048])
SLICE = 32512            # msgs rows per phase-2 slice (int16-safe)
CHUNK = 25088
NCORES_ENC = 4
BANK = 512               # psum bank cols (f32)


def cdiv(a, b):
    return (a + b - 1) // b


def rup(a, b):
    return cdiv(a, b) * b


def wrap16(a):
    w = a.reshape(-1, 16).T
    return np.tile(w, (8, 1)).copy()


def wrap128(a):
    return np.ascontiguousarray(a.reshape(-1, 128).T)


# ---------------------------------------------------------------------------
# host planning
# ---------------------------------------------------------------------------

def plan_encoder(x, edge_index, edge_type, batch, n_graphs):
    N = x.shape[0]
    src, dst = edge_index[0].astype(np.int64), edge_index[1].astype(np.int64)
    et = edge_type.astype(np.int64)
    gpc = n_graphs // NCORES_ENC
    bounds = [int(np.searchsorted(batch, c * gpc)) for c in range(NCORES_ENC)] + [N]
    key = dst * R + et
    uniq, inv, cnt = np.unique(key, return_inverse=True, return_counts=True)
    scale_all = (1.0 / cnt[inv]).astype(np.float32)
    cores = []
    for c in range(NCORES_ENC):
        ns, ne = bounds[c], bounds[c + 1]
        m = (dst >= ns) & (dst < ne)
        cores.append(dict(ns=ns, ne=ne, n_own=ne - ns,
                          src=src[m], dstl=(dst[m] - ns), et=et[m],
                          scale=scale_all[m]))
    return cores


def rank_within_sorted(keys_sorted):
    n = len(keys_sorted)
    if n == 0:
        return np.zeros(0, np.int64)
    first = np.r_[True, keys_sorted[1:] != keys_sorted[:-1]]
    starts = np.where(first)[0]
    sizes = np.diff(np.r_[starts, n])
    return np.arange(n) - np.repeat(starts, sizes)


def build_phase1(cores8, chunk_size, nchunks, parity_cells):
    """Common phase-1 schedule + per-core gidx streams and token dst/scale."""
    ncell = nchunks * parity_cells * R
    cells_per_chunk = parity_cells * R
    cell_of = []
    for co in cores8:
        chunk = co["tsrc"] // chunk_size
        cell_of.append((chunk * parity_cells + co["tpar"]) * R + co["et"])
    sizes = np.zeros(ncell, np.int64)
    for cell in cell_of:
        sizes = np.maximum(sizes, np.bincount(cell, minlength=ncell))
    sizes = rup(sizes, TOK)
    chunk_tok = rup(sizes.reshape(nchunks, cells_per_chunk).sum(1),
                    TOK * CALL_G)
    E_PAD = int(chunk_tok.sum())
    ncalls = E_PAD // (TOK * CALL_G)
    cell_off = np.zeros(ncell, np.int64)
    chunk_call0 = np.zeros(nchunks + 1, np.int64)
    off = 0
    for ch in range(nchunks):
        chunk_call0[ch] = off // (TOK * CALL_G)
        for cc in range(cells_per_chunk):
            i = ch * cells_per_chunk + cc
            cell_off[i] = off
            off += sizes[i]
        off = int(rup(off, TOK * CALL_G))
    chunk_call0[nchunks] = ncalls
    gtype = np.zeros(E_PAD // TOK, np.int64)
    for ch in range(nchunks):
        for cc in range(cells_per_chunk):
            i = ch * cells_per_chunk + cc
            o, s = int(cell_off[i]), int(sizes[i])
            gtype[o // TOK:(o + s) // TOK] = cc
    per_core = []
    for ci, co in enumerate(cores8):
        cell = cell_of[ci]
        order = np.argsort(cell, kind="stable")
        pos = cell_off[cell[order]] + rank_within_sorted(cell[order])
        gidx = np.zeros(E_PAD, np.int16)
        scale = np.zeros(E_PAD, np.float32)
        dstl = np.full(E_PAD, -1, np.int64)
        gidx[pos] = (co["tsrc"][order] % chunk_size).astype(np.int16)
        scale[pos] = co["scale"][order]
        dstl[pos] = co["dstl"][order]
        per_core.append(dict(gidx=gidx, scale=scale, dstl=dstl))
    return dict(E_PAD=E_PAD, ncalls=ncalls, gtype=gtype,
                chunk_call0=chunk_call0, per_core=per_core)


def build_phase2(p1, N_PAD):
    """Phase-2 schedule. Per (range, slice): per-core token lists sorted by
    dst; groups of 128 tokens; per group the distinct-dst count and psum
    col offset (= first dst - range base) must be SPMD-common -> we make the
    *group schedule* common by padding: for each (rg, sl) take the max group
    count over cores; per group, the indicator handles arbitrary per-core
    content (seg-in-group = dstl - group_first_dst, with group_first_dst a
    PER-CORE value baked into the seg stream; the psum col offset however is
    an instruction attribute!).

    To keep psum col offsets common across cores we use a fixed mapping:
    group g of (rg, sl) writes cols [0, RANGE_D) via indicator width d=128
    at col offset off_g where off_g is per-(rg,sl,g) COMMON. We achieve this
    by giving every group the FULL 128-wide indicator with seg-in-group =
    dstl - off_g - rg*RANGE_D, where off_g = common col offset. Host picks
    off_g = min over cores of (first dst in group) clamped so that all
    cores' dsts in group g fall in [off_g, off_g+128). This requires group
    contents aligned across cores -> instead we simply sort each core's
    tokens by dst and pick off_g = max(0, min(RANGE_D-128, g_quantile)):
    with tokens sorted by dst, group g's dsts lie near the g-th quantile of
    the range. Because token counts per (rg, sl) differ per core, quantiles
    differ -> we pad each (rg, sl) per-core token list to the common size
    first (pad tokens: scale 0, dst = off_g so they hit a valid col), THEN
    split into groups; off_g = max over cores of group-first-dst... still
    per-core spread. Resolution: indicator width 128 covers a 128-dst
    window; we pick off_g common = min over cores of first-dst; cores whose
    group spans beyond off_g+128 would lose tokens -> prevented by ALIGNING
    group boundaries to dst windows of 64: tokens are binned into windows of
    64 dsts; per (rg, sl, win64) the token count is padded to the common max
    (so group boundaries = window boundaries are common); groups within a
    window: indicator width 64 at col offset win*64 (common). Window token
    counts per (rg, sl, win) ~ Poisson(small) -> pad overhead high if wins
    are shredded... accept: win=64, tokens per (rg,sl,win) ~= 64*16.4/13
    ~= 81 -> padded max-over-8-cores ~= 81+4*9 ~= 117 -> rup128 -> 128:
    pad ~60%?? Too high. Use win = RANGE_D (one window per range, indicator
    128 wide can't cover 2048)... Final compromise: win = 128 dsts,
    indicator width 128, tokens per (rg, sl, win128) ~= 162 -> padded
    ~rup(162+50) = 256 vs 162: ~45% pad. Ouch but correct and simple.
    Actually choose win = 256 with TWO indicator mms per group (cols split
    at 128): tokens/win ~= 324 -> pad to ~rup(324+64,128)=512?? 30%.
    We go with win=128 for simplicity. pad measured at runtime.
    """
    E_PAD = p1["E_PAD"]
    nslice = cdiv(E_PAD, SLICE)
    nrange = cdiv(N_PAD, RANGE_D)
    WIND = 128
    nwin = RANGE_D // WIND
    # per (rg, sl, win) common padded token counts
    cnts = np.zeros((nrange, nslice, nwin), np.int64)
    toks = []
    for pc in p1["per_core"]:
        dstl = pc["dstl"]
        t = np.where(dstl >= 0)[0]
        rg = dstl[t] // RANGE_D
        sl = t // SLICE
        win = (dstl[t] % RANGE_D) // WIND
        c = np.zeros((nrange, nslice, nwin), np.int64)
        np.add.at(c, (rg, sl, win), 1)
        cnts = np.maximum(cnts, c)
        toks.append((t, rg, sl, win))
    cnts = rup(cnts, TOK)
    cnts[:, 0, :] = np.maximum(cnts[:, 0, :], TOK)
    # offsets in the phase-2 stream
    flat = cnts.reshape(-1)
    off = np.zeros(len(flat) + 1, np.int64)
    off[1:] = np.cumsum(flat)
    P2_PAD = int(off[-1])
    # static group schedule: for each (rg, sl): list of (win, ngroups)
    per_core = []
    for ci, pc in enumerate(p1["per_core"]):
        t, rg, sl, win = toks[ci]
        dstl = pc["dstl"]
        key = (rg * nslice + sl) * nwin + win
        order = np.argsort((key * (E_PAD + 1) + t), kind="stable")
        ts = t[order]
        ks = key[order]
        pos = off[ks] + rank_within_sorted(ks)
        g2 = np.zeros(P2_PAD, np.int16)
        seg = np.zeros(P2_PAD, np.float32)
        sc2 = np.zeros(P2_PAD, np.float32)
        g2[pos] = (ts % SLICE).astype(np.int16)
        seg[pos] = (dstl[ts] % WIND).astype(np.float32)
        sc2[pos] = pc["scale"][ts]
        per_core.append(dict(g2=g2, seg=seg, sc2=sc2))
    return dict(nslice=nslice, nrange=nrange, WIND=WIND, nwin=nwin,
                cnts=cnts, off=off, P2_PAD=P2_PAD, per_core=per_core)


# ---------------------------------------------------------------------------
# kernel builder
# ---------------------------------------------------------------------------

def build_kernel(cfg):
    N_PAD, NT1, NT2 = cfg["N_PAD"], cfg["NT1"], cfg["NT2"]
    L1, L2 = cfg["L1"], cfg["L2"]
    NG, NB = cfg["NG"], cfg["NB"]
    NGP = rup(max(NG, 64), 64)
    chunk2 = N_PAD // 2

    nc = bacc.Bacc(None, target_bir_lowering=False, debug=False)

    def din(name, shape, dt):
        return nc.declare_dram_parameter(name, list(shape), dt, isOutput=False)

    xtab = din("xtab", [NT1, IN], DT.bfloat16)
    xT = din("xT", [IN, N_PAD], DT.bfloat16)
    w1 = din("w1", [IN, R, H], DT.bfloat16)
    wr1 = din("wr1", [IN, H], DT.bfloat16)
    w2 = din("w2", [IN, 2 * R, H], DT.bfloat16)
    wr2 = din("wr2", [H, H], DT.bfloat16)
    b1c = din("b1c", [H, 1], DT.float32)
    b2c = din("b2c", [H, 1], DT.float32)
    gi1 = din("gi1", [128, L1["p1"]["E_PAD"] // 16], DT.int16)
    gi2 = din("gi2", [128, L2["p1"]["E_PAD"] // 16], DT.int16)
    p2g1 = din("p2g1", [128, L1["p2"]["P2_PAD"] // 16], DT.int16)
    p2g2 = din("p2g2", [128, L2["p2"]["P2_PAD"] // 16], DT.int16)
    seg1 = din("seg1", [128, L1["p2"]["P2_PAD"] // 128], DT.float32)
    seg2 = din("seg2", [128, L2["p2"]["P2_PAD"] // 128], DT.float32)
    sc1 = din("sc1", [128, L1["p2"]["P2_PAD"] // 128], DT.float32)
    sc2 = din("sc2", [128, L2["p2"]["P2_PAD"] // 128], DT.float32)
    pmat = din("pmat", [N_PAD // H, H, NG], DT.bfloat16)
    depth = din("depth", [1, NB], DT.float32)
    dep_own = din("dep_own", [1, NG], DT.float32)
    zsel = din("zsel", [128, 8], DT.int16)
    rw1 = din("rw1", [2 * H + 1, H], DT.float32)
    rb1 = din("rb1", [H, 1], DT.float32)
    rw2 = din("rw2", [H, 1], DT.float32)
    rb2c = din("rb2c", [1, 1], DT.float32)
    out = nc.declare_dram_parameter("out", [1, NG], DT.float32, isOutput=True)
    dbg = nc.declare_dram_parameter("dbg", [128, 128], DT.float32, isOutput=True)

    msgs1 = nc.dram_tensor("msgs1", [L1["p1"]["E_PAD"], H], DT.float32)
    msgs2 = nc.dram_tensor("msgs2", [L2["p1"]["E_PAD"], H], DT.float32)
    h_own = nc.dram_tensor("h_own", [N_PAD, H], DT.float32)
    htab_own = nc.dram_tensor("htab_own", [chunk2, IN], DT.bfloat16)
    htab = nc.dram_tensor("htab", [NT2, IN], DT.bfloat16)
    pool_own = nc.dram_tensor("pool_own", [H, NGP], DT.float32)
    pool_all = nc.dram_tensor("pool_all", [8 * H, NGP], DT.float32)

    with tile.TileContext(nc) as tc:
        with (
            tc.tile_pool(name="const", bufs=1) as constp,
            tc.tile_pool(name="work", bufs=3) as work,
            tc.tile_pool(name="res", bufs=1) as resp,
        ):
            w1_t = constp.tile([IN, R, H], DT.bfloat16)
            nc.sync.dma_start(w1_t[:], w1[:, :, :])
            w2_t = constp.tile([IN, 2 * R, H], DT.bfloat16)
            nc.sync.dma_start(w2_t[:], w2[:, :, :])
            wr1_t = constp.tile([IN, H], DT.bfloat16)
            nc.sync.dma_start(wr1_t[:], wr1[:, :])
            wr2_t = constp.tile([H, H], DT.bfloat16)
            nc.sync.dma_start(wr2_t[:], wr2[:, :])
            b1_t = constp.tile([H, 1], DT.float32)
            nc.sync.dma_start(b1_t[:], b1c[:, :])
            b2_t = constp.tile([H, 1], DT.float32)
            nc.sync.dma_start(b2_t[:], b2c[:, :])
            iota128 = constp.tile([128, 128], DT.float32)
            nc.gpsimd.iota(iota128[:], [[1, 128]], channel_multiplier=0, allow_small_or_imprecise_dtypes=True)
            iotap = constp.tile([64, 1], DT.float32)
            nc.gpsimd.iota(iotap[:], [[1, 1]], channel_multiplier=1, allow_small_or_imprecise_dtypes=True)
            ident = constp.tile([64, 64], DT.bfloat16)
            nc.vector.tensor_scalar(ident[:], iota128[0:64, 0:64],
                                    iotap[:], None, AluOpType.is_equal)

            acc = resp.tile([H, N_PAD], DT.bfloat16)
            hT = resp.tile([H, N_PAD], DT.bfloat16)

            def run_layer(L, gidx_p, tabl, csize, wstack, p2g_p, seg_p, sc_p,
                          msgs_buf):
                p1, p2 = L["p1"], L["p2"]
                E_PAD, ncalls = p1["E_PAD"], p1["ncalls"]
                gtype, chunk_call0 = p1["gtype"], p1["chunk_call0"]
                # ---------- phase 1 ----------
                with (
                    tc.tile_pool(name="p1b", bufs=2) as p1b,
                    tc.tile_pool(name="p1ps", bufs=2, space="PSUM") as p1ps,
                ):
                    for call in range(min(ncalls, MAXCALL)):
                        ch = int(np.searchsorted(chunk_call0, call,
                                                 side="right") - 1)
                        t0 = call * TOK * CALL_G
                        gidx_t = p1b.tile([128, TOK * CALL_G // 16], DT.int16,
                                          tag="gidx")
                        nc.sync.dma_start(
                            gidx_t[:],
                            gidx_p[:, t0 // 16:(t0 + TOK * CALL_G) // 16])
                        xgt = p1b.tile([128, 1, TOK * CALL_G], DT.bfloat16,
                                       tag="xgt")
                        if SUB & 1:
                            nc.gpsimd.dma_gather(
                                xgt[:], tabl[ch * csize:(ch + 1) * csize, :],
                                gidx_t[:], TOK * CALL_G, TOK * CALL_G, IN,
                                transpose=True, single_packet=False)
                        else:
                            nc.gpsimd.memset(xgt[:], 0.0)
                        ps = p1ps.tile([128, CALL_G, H], DT.float32,
                                       tag="p1ps")
                        if SUB & 2:
                            for g in range(CALL_G):
                                cc = int(gtype[call * CALL_G + g])
                                nc.tensor.matmul(
                                    ps[:, g, :],
                                    xgt[:, 0, g * TOK:(g + 1) * TOK],
                                    wstack[:, cc, :], start=True, stop=True)
                        mt = p1b.tile([128, CALL_G, H], DT.float32, tag="p1m")
                        if SUB & 4:
                            nc.scalar.activation(
                                mt[:], ps[:],
                                mybir.ActivationFunctionType.Copy)
                        else:
                            nc.gpsimd.memset(mt[:], 0.0)
                        if SUB & 8:
                            nc.sync.dma_start(
                                msgs_buf[t0:t0 + TOK * CALL_G, :].rearrange(
                                    "(g p) h -> p g h", p=128), mt[:])
                # ---------- phase 2 ----------
                if STAGE < 2 + (0 if msgs_buf is msgs1 else 4):
                    return
                nrange, nslice = p2["nrange"], p2["nslice"]
                WIND, nwin = p2["WIND"], p2["nwin"]
                cnts, off = p2["cnts"], p2["off"]
                with (
                    tc.tile_pool(name="p2b", bufs=2) as p2b,
                    tc.tile_pool(name="p2ps", bufs=2, space="PSUM") as p2ps,
                ):
                    for rg in range(nrange):
                        nd = min(RANGE_D, N_PAD - rg * RANGE_D)
                        ps2 = p2ps.tile([H, RANGE_D], DT.float32, tag="p2ps")
                        for sl in range(nslice):
                            ntok = int(cnts[rg, sl].sum())
                            if ntok == 0:
                                continue
                            o = int(off[(rg * nslice + sl) * nwin])
                            g2_t = p2b.tile([128, ntok // 16], DT.int16,
                                            tag="g2")
                            nc.sync.dma_start(
                                g2_t[:], p2g_p[:, o // 16:(o + ntok) // 16])
                            sbase = sl * SLICE
                            ssize = min(SLICE, E_PAD - sbase)
                            mg = p2b.tile([128, ntok // 128, H], DT.float32,
                                          tag="mg")
                            for a in range(0, ntok, GMAX):
                                nb = min(GMAX, ntok - a)
                                nc.gpsimd.dma_gather(
                                    mg[:, a // 128:(a + nb) // 128, :],
                                    msgs_buf[sbase:sbase + ssize, :],
                                    g2_t[:, a // 16:(a + nb) // 16],
                                    nb, nb, H, single_packet=False)
                            sc_t = p2b.tile([128, ntok // 128], DT.float32,
                                            tag="scs")
                            nc.sync.dma_start(
                                sc_t[:], sc_p[:, o // 128:(o + ntok) // 128])
                            mgb = p2b.tile([128, ntok // 128, H],
                                           DT.bfloat16, tag="mgb")
                            nc.vector.tensor_tensor(
                                mgb[:], mg[:],
                                sc_t[:, :, None].broadcast_to(
                                    [128, ntok // 128, H]),
                                AluOpType.mult)
                            sg_t = p2b.tile([128, ntok // 128], DT.float32,
                                            tag="sgs")
                            nc.sync.dma_start(
                                sg_t[:], seg_p[:, o // 128:(o + ntok) // 128])
                            ind = p2b.tile([128, ntok // 128, WIND],
                                           DT.bfloat16, tag="ind")
                            nc.vector.tensor_tensor(
                                ind[:],
                                iota128[:, None, 0:WIND].broadcast_to(
                                    [128, ntok // 128, WIND]),
                                sg_t[:, :, None].broadcast_to(
                                    [128, ntok // 128, WIND]),
                                AluOpType.is_equal)
                            gg = 0
                            for w in range(nwin):
                                ngr = int(cnts[rg, sl, w]) // TOK
                                col0 = w * WIND
                                for k in range(ngr):
                                    st = (sl == 0 and k == 0 and w % 4 == 0)
                                    nc.tensor.matmul(
                                        ps2[:, col0:col0 + WIND],
                                        mgb[:, gg, :],
                                        ind[:, gg, :],
                                        start=st, stop=False,
                                        skip_group_check=True)
                                    gg += 1
                        oc = p2b.tile([H, RANGE_D], DT.bfloat16, tag="oc")
                        nc.vector.tensor_copy(oc[:], ps2[:])
                        nc.vector.tensor_copy(
                            acc[:, rg * RANGE_D:rg * RANGE_D + nd],
                            oc[:, :nd])

            def epilogue(layer, wr_t, b_t, pool_ps):
                ntile = N_PAD // H
                with (
                    tc.tile_pool(name="epb", bufs=3) as epb,
                    tc.tile_pool(name="epps", bufs=2, space="PSUM") as epps,
                ):
                    for t in range(ntile):
                        rt = epps.tile([H, H], DT.float32, tag="root")
                        if layer == 1:
                            xtt = epb.tile([IN, H], DT.bfloat16, tag="xtt")
                            nc.sync.dma_start(xtt[:],
                                              xT[:, t * H:(t + 1) * H])
                            nc.tensor.matmul(rt[:], wr_t[:], xtt[:],
                                             start=True, stop=True)
                        else:
                            nc.tensor.matmul(rt[:], wr_t[:],
                                             hT[:, t * H:(t + 1) * H],
                                             start=True, stop=True)
                        af = epb.tile([H, H], DT.float32, tag="af")
                        nc.vector.tensor_copy(af[:],
                                              acc[:, t * H:(t + 1) * H])
                        hh = epb.tile([H, H], DT.float32, tag="hh")
                        nc.vector.tensor_tensor(hh[:], rt[:], af[:],
                                                AluOpType.add)
                        hrT = epb.tile([H, H], DT.bfloat16, tag="hrT")
                        nc.scalar.activation(hrT[:], hh[:],
                                             mybir.ActivationFunctionType.Relu,
                                             bias=b_t[:])
                        if layer == 1:
                            nc.vector.tensor_copy(hT[:, t * H:(t + 1) * H],
                                                  hrT[:])
                            # node-major rows for the gather table
                            tp = epps.tile([H, H], DT.bfloat16, tag="tp")
                            nc.tensor.transpose(tp[:], hrT[:], ident[:])
                            hrow = epb.tile([H, H], DT.float32, tag="hrow")
                            nc.vector.tensor_copy(hrow[:], tp[:])
                            nc.sync.dma_start(h_own[t * H:(t + 1) * H, :],
                                              hrow[:])
                        else:
                            # pool: out [h, g] += h2_tile(node-major) @ P
                            tp = epps.tile([H, H], DT.bfloat16, tag="tp")
                            nc.tensor.transpose(tp[:], hrT[:], ident[:])
                            h2n = epb.tile([H, H], DT.bfloat16, tag="h2n")
                            nc.vector.tensor_copy(h2n[:], tp[:])
                            pm = epb.tile([H, NG], DT.bfloat16, tag="pm")
                            nc.sync.dma_start(pm[:], pmat[t, :, :])
                            nc.tensor.matmul(pool_ps[:], h2n[:], pm[:],
                                             start=(t == 0),
                                             stop=(t == ntile - 1),
                                             skip_group_check=True)

            def dump_done(src_ap, cast=False):
                d = work.tile([128, 128], DT.float32, tag="dbgt")
                nc.gpsimd.memset(d[:], 0.0)
                if src_ap is not None:
                    if cast:
                        nc.vector.tensor_copy(d[:src_ap.shape[0], :src_ap.shape[-1]], src_ap)
                    else:
                        nc.sync.dma_start(d[:src_ap.shape[0], :src_ap.shape[-1]], src_ap)
                nc.sync.dma_start(dbg[:, :], d[:])
                o = work.tile([1, NG], DT.float32, tag="ot0")
                nc.gpsimd.memset(o[:], 0.0)
                nc.sync.dma_start(out[:, :], o[:])

            if STAGE >= 1:
                run_layer(L1, gi1, xtab, CHUNK, w1_t, p2g1, seg1, sc1, msgs1)
            if STAGE == 0:
                dump_done(None)
            elif STAGE == 1:
                dump_done(msgs1[0:128, :])
            elif STAGE == 2:
                dump_done(acc[:, 0:128], cast=True)
            if STAGE <= 2:
                pass
            else:
                epilogue(1, wr1_t, b1_t, None)
            if STAGE == 3:
                dump_done(hT[:64, 0:128], cast=True)
            if STAGE > 3:
                nc.gpsimd.dma_start(
                    out=htab_own[:, :],
                    in_=h_own[:, :].rearrange("(a b) h -> a (b h)", b=2))
                nc.gpsimd.collective_compute(
                    "AllGather", mybir.AluOpType.bypass,
                    replica_groups=[[0, 1, 2, 3], [4, 5, 6, 7]],
                    ins=[htab_own[:, :]], outs=[htab[:, :]])
            if STAGE == 4:
                dump_done(None)
            if STAGE > 4:
                run_layer(L2, gi2, htab, chunk2, w2_t, p2g2, seg2, sc2, msgs2)
            if STAGE == 5:
                dump_done(None)
            elif STAGE == 6:
                dump_done(acc[:, 0:128], cast=True)
            if STAGE < 7:
                ctx_skip = True
            else:
                ctx_skip = False
            with tc.tile_pool(name="plps", bufs=1, space="PSUM") as plps:
              if not ctx_skip:
                pool_ps = plps.tile([H, NG], DT.float32)
                epilogue(2, wr2_t, b2_t, pool_ps)
                with (
                    tc.tile_pool(name="fin", bufs=1) as fin,
                    tc.tile_pool(name="fps", bufs=1, space="PSUM") as fps,
                ):
                    pool_t = fin.tile([H, NGP], DT.float32)
                    nc.gpsimd.memset(pool_t[:], 0.0)
                    nc.vector.tensor_copy(pool_t[:, :NG], pool_ps[:])
                    nc.sync.dma_start(pool_own[:, :], pool_t[:])
                    nc.gpsimd.collective_compute(
                        "AllGather", mybir.AluOpType.bypass,
                        replica_groups=[[0, 1, 2, 3, 4, 5, 6, 7]],
                        ins=[pool_own[:, :]], outs=[pool_all[:, :]])
                    zsel_t = fin.tile([128, 8], DT.int16)
                    nc.sync.dma_start(zsel_t[:], zsel[:, :])
                    zt3 = fin.tile([128, 1, NGP], DT.float32)
                    nc.gpsimd.dma_gather(zt3[:], pool_all[:, :], zsel_t[:],
                                         128, 128, NGP)
                    zt = zt3[:, 0, :NG]
                    # depth normalization (global stats, own slice)
                    dep = fin.tile([1, NB], DT.float32)
                    nc.sync.dma_start(dep[:], depth[:, :])
                    dpo = fin.tile([1, NG], DT.float32)
                    nc.sync.dma_start(dpo[:], dep_own[:, :])
                    mean = fin.tile([1, 1], DT.float32)
                    nc.vector.tensor_reduce(mean[:], dep[:],
                                            mybir.AxisListType.X,
                                            AluOpType.add)
                    nc.vector.tensor_scalar(mean[:], mean[:], 1.0 / NB, None,
                                            AluOpType.mult)
                    cen = fin.tile([1, NB], DT.float32)
                    nc.vector.tensor_scalar(cen[:], dep[:], mean[:, 0:1],
                                            None, AluOpType.subtract)
                    sq = fin.tile([1, NB], DT.float32)
                    nc.vector.tensor_tensor(sq[:], cen[:], cen[:],
                                            AluOpType.mult)
                    var = fin.tile([1, 1], DT.float32)
                    nc.vector.tensor_reduce(var[:], sq[:],
                                            mybir.AxisListType.X,
                                            AluOpType.add)
                    nc.vector.tensor_scalar(var[:], var[:], 1.0 / NB, None,
                                            AluOpType.mult)
                    std = fin.tile([1, 1], DT.float32)
                    nc.scalar.activation(std[:], var[:],
                                         mybir.ActivationFunctionType.Sqrt)
                    nc.vector.tensor_scalar(std[:], std[:], 1e-6, None,
                                            AluOpType.add)
                    istd = fin.tile([1, 1], DT.float32)
                    nc.vector.reciprocal(istd[:], std[:])
                    dno = fin.tile([1, NG], DT.float32)
                    nc.vector.tensor_scalar(dno[:], dpo[:], mean[:, 0:1],
                                            None, AluOpType.subtract)
                    nc.vector.tensor_scalar(dno[:], dno[:], istd[:, 0:1],
                                            None, AluOpType.mult)
                    # regressor
                    rw1_t = fin.tile([2 * H, H], DT.float32)
                    nc.sync.dma_start(rw1_t[:], rw1[0:2 * H, :])
                    rw1r = fin.tile([1, H], DT.float32)
                    nc.sync.dma_start(rw1r[:], rw1[2 * H:2 * H + 1, :])
                    rb1_t = fin.tile([H, 1], DT.float32)
                    nc.sync.dma_start(rb1_t[:], rb1[:, :])
                    rw2_t = fin.tile([H, 1], DT.float32)
                    nc.sync.dma_start(rw2_t[:], rw2[:, :])
                    rb2_t = fin.tile([1, 1], DT.float32)
                    nc.sync.dma_start(rb2_t[:], rb2c[:, :])
                    r1 = fps.tile([H, NG], DT.float32)
                    nc.tensor.matmul(r1[:], rw1_t[:], zt[:],
                                     start=True, stop=False,
                                     skip_group_check=True)
                    nc.tensor.matmul(r1[:], rw1r[:], dno[:],
                                     start=False, stop=True,
                                     skip_group_check=True)
                    r1s = fin.tile([H, NG], DT.float32)
                    nc.scalar.activation(r1s[:], r1[:],
                                         mybir.ActivationFunctionType.Relu,
                                         bias=rb1_t[:])
                    r2 = fps.tile([1, NG], DT.float32)
                    nc.tensor.matmul(r2[:], rw2_t[:], r1s[:], start=True,
                                     stop=True)
                    ot = fin.tile([1, NG], DT.float32)
                    nc.vector.tensor_scalar(ot[:], r2[:], rb2_t[:, 0:1],
                                            None, AluOpType.add)
                    nc.sync.dma_start(out[:, :], ot[:])

    nc.compile()
    return nc


# ---------------------------------------------------------------------------
# entry point
# ---------------------------------------------------------------------------

def kernel(**inputs):
    inp = {k: np.asarray(v) for k, v in inputs.items()}
    N = inp["state_x"].shape[0]
    NB = inp["depth"].shape[0]
    NG = NB // NCORES_ENC

    st = plan_encoder(inp["state_x"], inp["state_edge_index"],
                      inp["state_edge_type"], inp["state_batch"], NB)
    go = plan_encoder(inp["goal_x"], inp["goal_edge_index"],
                      inp["goal_edge_type"], inp["goal_batch"], NB)
    cores = st + go
    N_PAD = rup(max(c["n_own"] for c in cores), RANGE_D)
    NT1 = rup(N, CHUNK)
    nchunks1 = NT1 // CHUNK
    chunk2 = N_PAD // 2
    NT2 = NCORES_ENC * chunk2

    for c in cores:
        c["tsrc"] = c["src"]
        c["tpar"] = np.zeros(len(c["src"]), np.int64)
    p1_1 = build_phase1(cores, CHUNK, nchunks1, 1)
    p2_1 = build_phase2(p1_1, N_PAD)

    for enc, grp in ((0, st), (1, go)):
        nsb = np.array([c["ns"] for c in grp] + [N], np.int64)
        for c in grp:
            owner = np.searchsorted(nsb, c["src"], side="right") - 1
            loc = c["src"] - nsb[owner]
            c["tsrc"] = owner * chunk2 + loc // 2
            c["tpar"] = loc % 2
    p1_2 = build_phase1(cores, chunk2, NCORES_ENC, 2)
    p2_2 = build_phase2(p1_2, N_PAD)

    L1 = dict(p1=p1_1, p2=p2_1)
    L2 = dict(p1=p1_2, p2=p2_2)
    cfg = dict(N_PAD=N_PAD, NT1=NT1, NT2=NT2, L1=L1, L2=L2, NG=NG, NB=NB)
    nc = build_kernel(cfg)

    in_maps = []
    for ci in range(8):
        enc = ci // NCORES_ENC
        c = cores[ci]
        pfx = "s" if enc == 0 else "g"
        x = inp["state_x"] if enc == 0 else inp["goal_x"]
        batch = inp["state_batch"] if enc == 0 else inp["goal_batch"]
        xt = np.zeros((NT1, IN), BF16)
        xt[:N] = x.astype(BF16)
        xTv = np.zeros((IN, N_PAD), BF16)
        xTv[:, :c["n_own"]] = x[c["ns"]:c["ne"]].T.astype(BF16)
        W1 = np.ascontiguousarray(
            inp[pfx + "W1"].astype(BF16).transpose(1, 0, 2))
        W2s = np.zeros((IN, 2 * R, H), BF16)
        W2s[:H, :R] = inp[pfx + "W2"].astype(BF16).transpose(1, 0, 2)
        W2s[H:2 * H, R:] = inp[pfx + "W2"].astype(BF16).transpose(1, 0, 2)
        g0 = (ci % NCORES_ENC) * NG
        bown = batch[c["ns"]:c["ne"]] - g0
        gcnt = np.maximum(
            np.bincount(bown, minlength=NG).astype(np.float32), 1.0)
        pm = np.zeros((N_PAD // H, H, NG), BF16)
        nidx = np.arange(c["n_own"])
        pm[nidx // H, nidx % H, bown] = (1.0 / gcnt[bown]).astype(BF16)
        im = dict(
            xtab=xt, xT=xTv,
            w1=W1, wr1=inp[pfx + "Root1"].astype(BF16),
            w2=W2s, wr2=inp[pfx + "Root2"].astype(BF16),
            b1c=inp[pfx + "B1"].astype(np.float32)[:, None],
            b2c=inp[pfx + "B2"].astype(np.float32)[:, None],
            gi1=wrap16(p1_1["per_core"][ci]["gidx"]),
            gi2=wrap16(p1_2["per_core"][ci]["gidx"]),
            p2g1=wrap16(p2_1["per_core"][ci]["g2"]),
            p2g2=wrap16(p2_2["per_core"][ci]["g2"]),
            seg1=wrap128(p2_1["per_core"][ci]["seg"]),
            seg2=wrap128(p2_2["per_core"][ci]["seg"]),
            sc1=wrap128(p2_1["per_core"][ci]["sc2"]),
            sc2=wrap128(p2_2["per_core"][ci]["sc2"]),
            pmat=pm,
            depth=inp["depth"].astype(np.float32)[None, :],
            dep_own=inp["depth"].astype(np.float32)[None, g0:g0 + NG],
            rw1=inp["rW1"].astype(np.float32),
            rb1=inp["rB1"].astype(np.float32)[:, None],
            rw2=inp["rW2"].astype(np.float32),
            rb2c=inp["rB2"].astype(np.float32)[None, :],
            zsel=wrap16(np.r_[(ci % NCORES_ENC) * H + np.arange(H),
                              ((ci % NCORES_ENC) + NCORES_ENC) * H
                              + np.arange(H)].astype(np.int16)),
        )
        in_maps.append(im)

    res = bass_utils.run_bass_kernel_spmd(nc, in_maps,
                                          core_ids=list(range(8)))
    global LAST_DBG, LAST_PLANS, LAST_RES
    LAST_RES = res
    LAST_DBG = [res.results[c].get("dbg") for c in range(8)]
    LAST_PLANS = (cores, L1, L2, cfg)
    outp = np.zeros(NB, np.float32)
    for ci in range(NCORES_ENC):
        outp[ci * NG:(ci + 1) * NG] = res.results[ci]["out"][0]
    return outp



# revision 6
# speedup vs baseline: 1.1596x; 1.1596x over previous
"""Distance-estimator (2x RGCN encoder + regressor) on 8 TRN2 NeuronCores.

Sharding: cores 0-3 = state encoder, 4-7 = goal encoder (SPMD, different
data). Each core owns a contiguous quarter of the graphs -> contiguous
dst-node range (batch sorted). Per core-layer:

Phase 1: edges sorted by (src-chunk, type[, parity]); dma_gather source rows
  (bf16, transposed) from the feature table; per-128-token matmul with W_type
  -> msgs [tok, 64] f32 written sequentially to an HBM buffer.
Phase 2: msgs re-gathered in (dst-range, msgs-slice) order: for each range
  of 2048 dst nodes x each 32k-row msgs slice (int16 idx), gather the
  tokens, scale by 1/cnt(dst,type), aggregate with indicator matmuls
  (lhsT = one-hot of dst-within-group built via iota/is_equal) writing
  dense node-columns of a PSUM [64, 2048] accumulator -> acc [64h, N_PAD]
  bf16 image (transposed orientation).
Epilogue (transposed): accT + Wroot^T@xT + b -> relu -> hT tiles; L1 emits
  the pair-packed bf16 h table (AllGather across the encoder's 4 cores);
  L2 pools via matmul with host P matrices; paired cores exchange pooled
  embeddings; per-core f32 regressor for its 128 graphs.
"""
import os
import sys
sys.path.insert(0, "/opt/trn_rl_repo")
STAGE = int(os.environ.get("K_STAGE", "99"))
SUB = int(os.environ.get("K_SUB", "15"))
MAXCALL = int(os.environ.get("K_MAXCALL", "100000"))
import numpy as np
import ml_dtypes

from concourse import bass, bacc, tile
from concourse import bass_utils
from concourse.bass import mybir
from concourse.alu_op_type import AluOpType

DT = mybir.dt
BF16 = ml_dtypes.bfloat16
LAST_DBG = None
LAST_PLANS = None
LAST_RES = None

IN = 128
H = 64
R = 32
TOK = 128
CALL_G = int(os.environ.get("K_CALLG", "16"))
GMAX = 2048              # max idxs per dma_gather call
RANGE_D = 2048           # dsts per phase-2 range (psum [64, 2048])
SLICE = 32512            # msgs rows per phase-2 slice (int16-safe)
CHUNK = 25088
NCORES_ENC = 4
BANK = 512               # psum bank cols (f32)


def cdiv(a, b):
    return (a + b - 1) // b


def rup(a, b):
    return cdiv(a, b) * b


def wrap16(a):
    w = a.reshape(-1, 16).T
    return np.tile(w, (8, 1)).copy()


def wrap128(a):
    return np.ascontiguousarray(a.reshape(-1, 128).T)


# ---------------------------------------------------------------------------
# host planning
# ---------------------------------------------------------------------------

def plan_encoder(x, edge_index, edge_type, batch, n_graphs):
    N = x.shape[0]
    src, dst = edge_index[0].astype(np.int64), edge_index[1].astype(np.int64)
    et = edge_type.astype(np.int64)
    gpc = n_graphs // NCORES_ENC
    bounds = [int(np.searchsorted(batch, c * gpc)) for c in range(NCORES_ENC)] + [N]
    key = dst * R + et
    uniq, inv, cnt = np.unique(key, return_inverse=True, return_counts=True)
    scale_all = (1.0 / cnt[inv]).astype(np.float32)
    cores = []
    for c in range(NCORES_ENC):
        ns, ne = bounds[c], bounds[c + 1]
        m = (dst >= ns) & (dst < ne)
        cores.append(dict(ns=ns, ne=ne, n_own=ne - ns,
                          src=src[m], dstl=(dst[m] - ns), et=et[m],
                          scale=scale_all[m]))
    return cores


def rank_within_sorted(keys_sorted):
    n = len(keys_sorted)
    if n == 0:
        return np.zeros(0, np.int64)
    first = np.r_[True, keys_sorted[1:] != keys_sorted[:-1]]
    starts = np.where(first)[0]
    sizes = np.diff(np.r_[starts, n])
    return np.arange(n) - np.repeat(starts, sizes)


def build_phase1(cores8, chunk_size, nchunks, parity_cells):
    """Common phase-1 schedule + per-core gidx streams and token dst/scale."""
    ncell = nchunks * parity_cells * R
    cells_per_chunk = parity_cells * R
    cell_of = []
    for co in cores8:
        chunk = co["tsrc"] // chunk_size
        cell_of.append((chunk * parity_cells + co["tpar"]) * R + co["et"])
    sizes = np.zeros(ncell, np.int64)
    for cell in cell_of:
        sizes = np.maximum(sizes, np.bincount(cell, minlength=ncell))
    sizes = rup(sizes, TOK)
    chunk_tok = rup(sizes.reshape(nchunks, cells_per_chunk).sum(1),
                    TOK * CALL_G)
    E_PAD = int(chunk_tok.sum())
    ncalls = E_PAD // (TOK * CALL_G)
    cell_off = np.zeros(ncell, np.int64)
    chunk_call0 = np.zeros(nchunks + 1, np.int64)
    off = 0
    for ch in range(nchunks):
        chunk_call0[ch] = off // (TOK * CALL_G)
        for cc in range(cells_per_chunk):
            i = ch * cells_per_chunk + cc
            cell_off[i] = off
            off += sizes[i]
        off = int(rup(off, TOK * CALL_G))
    chunk_call0[nchunks] = ncalls
    gtype = np.zeros(E_PAD // TOK, np.int64)
    for ch in range(nchunks):
        for cc in range(cells_per_chunk):
            i = ch * cells_per_chunk + cc
            o, s = int(cell_off[i]), int(sizes[i])
            gtype[o // TOK:(o + s) // TOK] = cc
    per_core = []
    for ci, co in enumerate(cores8):
        cell = cell_of[ci]
        order = np.argsort(cell, kind="stable")
        pos = cell_off[cell[order]] + rank_within_sorted(cell[order])
        gidx = np.zeros(E_PAD, np.int16)
        scale = np.zeros(E_PAD, np.float32)
        dstl = np.full(E_PAD, -1, np.int64)
        gidx[pos] = (co["tsrc"][order] % chunk_size).astype(np.int16)
        scale[pos] = co["scale"][order]
        dstl[pos] = co["dstl"][order]
        per_core.append(dict(gidx=gidx, scale=scale, dstl=dstl))
    return dict(E_PAD=E_PAD, ncalls=ncalls, gtype=gtype,
                chunk_call0=chunk_call0, per_core=per_core)


def build_phase2(p1, N_PAD):
    """Phase-2 schedule. Per (range, slice): per-core token lists sorted by
    dst; groups of 128 tokens; per group the distinct-dst count and psum
    col offset (= first dst - range base) must be SPMD-common -> we make the
    *group schedule* common by padding: for each (rg, sl) take the max group
    count over cores; per group, the indicator handles arbitrary per-core
    content (seg-in-group = dstl - group_first_dst, with group_first_dst a
    PER-CORE value baked into the seg stream; the psum col offset however is
    an instruction attribute!).

    To keep psum col offsets common across cores we use a fixed mapping:
    group g of (rg, sl) writes cols [0, RANGE_D) via indicator width d=128
    at col offset off_g where off_g is per-(rg,sl,g) COMMON. We achieve this
    by giving every group the FULL 128-wide indicator with seg-in-group =
    dstl - off_g - rg*RANGE_D, where off_g = common col offset. Host picks
    off_g = min over cores of (first dst in group) clamped so that all
    cores' dsts in group g fall in [off_g, off_g+128). This requires group
    contents aligned across cores -> instead we simply sort each core's
    tokens by dst and pick off_g = max(0, min(RANGE_D-128, g_quantile)):
    with tokens sorted by dst, group g's dsts lie near the g-th quantile of
    the range. Because token counts per (rg, sl) differ per core, quantiles
    differ -> we pad each (rg, sl) per-core token list to the common size
    first (pad tokens: scale 0, dst = off_g so they hit a valid col), THEN
    split into groups; off_g = max over cores of group-first-dst... still
    per-core spread. Resolution: indicator width 128 covers a 128-dst
    window; we pick off_g common = min over cores of first-dst; cores whose
    group spans beyond off_g+128 would lose tokens -> prevented by ALIGNING
    group boundaries to dst windows of 64: tokens are binned into windows of
    64 dsts; per (rg, sl, win64) the token count is padded to the common max
    (so group boundaries = window boundaries are common); groups within a
    window: indicator width 64 at col offset win*64 (common). Window token
    counts per (rg, sl, win) ~ Poisson(small) -> pad overhead high if wins
    are shredded... accept: win=64, tokens per (rg,sl,win) ~= 64*16.4/13
    ~= 81 -> padded max-over-8-cores ~= 81+4*9 ~= 117 -> rup128 -> 128:
    pad ~60%?? Too high. Use win = RANGE_D (one window per range, indicator
    128 wide can't cover 2048)... Final compromise: win = 128 dsts,
    indicator width 128, tokens per (rg, sl, win128) ~= 162 -> padded
    ~rup(162+50) = 256 vs 162: ~45% pad. Ouch but correct and simple.
    Actually choose win = 256 with TWO indicator mms per group (cols split
    at 128): tokens/win ~= 324 -> pad to ~rup(324+64,128)=512?? 30%.
    We go with win=128 for simplicity. pad measured at runtime.
    """
    E_PAD = p1["E_PAD"]
    nslice = cdiv(E_PAD, SLICE)
    nrange = cdiv(N_PAD, RANGE_D)
    WIND = 512
    nwin = RANGE_D // WIND
    # per (rg, sl, win) common padded token counts
    cnts = np.zeros((nrange, nslice, nwin), np.int64)
    toks = []
    for pc in p1["per_core"]:
        dstl = pc["dstl"]
        t = np.where(dstl >= 0)[0]
        rg = dstl[t] // RANGE_D
        sl = t // SLICE
        win = (dstl[t] % RANGE_D) // WIND
        c = np.zeros((nrange, nslice, nwin), np.int64)
        np.add.at(c, (rg, sl, win), 1)
        cnts = np.maximum(cnts, c)
        toks.append((t, rg, sl, win))
    cnts = rup(cnts, TOK)
    cnts[:, 0, :] = np.maximum(cnts[:, 0, :], TOK)
    # offsets in the phase-2 stream
    flat = cnts.reshape(-1)
    off = np.zeros(len(flat) + 1, np.int64)
    off[1:] = np.cumsum(flat)
    P2_PAD = int(off[-1])
    # static group schedule: for each (rg, sl): list of (win, ngroups)
    per_core = []
    for ci, pc in enumerate(p1["per_core"]):
        t, rg, sl, win = toks[ci]
        dstl = pc["dstl"]
        key = (rg * nslice + sl) * nwin + win
        order = np.argsort((key * (E_PAD + 1) + t), kind="stable")
        ts = t[order]
        ks = key[order]
        pos = off[ks] + rank_within_sorted(ks)
        g2 = np.zeros(P2_PAD, np.int16)
        seg = np.zeros(P2_PAD, np.float32)
        sc2 = np.zeros(P2_PAD, np.float32)
        g2[pos] = (ts % SLICE).astype(np.int16)
        seg[pos] = (dstl[ts] % WIND).astype(np.float32)
        sc2[pos] = pc["scale"][ts]
        per_core.append(dict(g2=g2, seg=seg, sc2=sc2))
    return dict(nslice=nslice, nrange=nrange, WIND=WIND, nwin=nwin,
                cnts=cnts, off=off, P2_PAD=P2_PAD, per_core=per_core)


# ---------------------------------------------------------------------------
# kernel builder
# ---------------------------------------------------------------------------

def build_kernel(cfg):
    N_PAD, NT1, NT2 = cfg["N_PAD"], cfg["NT1"], cfg["NT2"]
    L1, L2 = cfg["L1"], cfg["L2"]
    NG, NB = cfg["NG"], cfg["NB"]
    NGP = rup(max(NG, 64), 64)
    chunk2 = N_PAD // 2

    nc = bacc.Bacc(None, target_bir_lowering=False, debug=False)

    def din(name, shape, dt):
        return nc.declare_dram_parameter(name, list(shape), dt, isOutput=False)

    xtab = din("xtab", [NT1, IN], DT.bfloat16)
    xT = din("xT", [IN, N_PAD], DT.bfloat16)
    w1 = din("w1", [IN, R, H], DT.bfloat16)
    wr1 = din("wr1", [IN, H], DT.bfloat16)
    w2 = din("w2", [IN, 2 * R, H], DT.bfloat16)
    wr2 = din("wr2", [H, H], DT.bfloat16)
    b1c = din("b1c", [H, 1], DT.float32)
    b2c = din("b2c", [H, 1], DT.float32)
    gi1 = din("gi1", [128, L1["p1"]["E_PAD"] // 16], DT.int16)
    gi2 = din("gi2", [128, L2["p1"]["E_PAD"] // 16], DT.int16)
    p2g1 = din("p2g1", [128, L1["p2"]["P2_PAD"] // 16], DT.int16)
    p2g2 = din("p2g2", [128, L2["p2"]["P2_PAD"] // 16], DT.int16)
    seg1 = din("seg1", [128, L1["p2"]["P2_PAD"] // 128], DT.float32)
    seg2 = din("seg2", [128, L2["p2"]["P2_PAD"] // 128], DT.float32)
    sc1 = din("sc1", [128, L1["p2"]["P2_PAD"] // 128], DT.float32)
    sc2 = din("sc2", [128, L2["p2"]["P2_PAD"] // 128], DT.float32)
    pmat = din("pmat", [N_PAD // H, H, NG], DT.bfloat16)
    depth = din("depth", [1, NB], DT.float32)
    dep_own = din("dep_own", [1, NG], DT.float32)
    zsel = din("zsel", [128, 8], DT.int16)
    rw1 = din("rw1", [2 * H + 1, H], DT.float32)
    rb1 = din("rb1", [H, 1], DT.float32)
    rw2 = din("rw2", [H, 1], DT.float32)
    rb2c = din("rb2c", [1, 1], DT.float32)
    out = nc.declare_dram_parameter("out", [1, NG], DT.float32, isOutput=True)
    dbg = nc.declare_dram_parameter("dbg", [128, 128], DT.float32, isOutput=True)

    msgs1 = nc.dram_tensor("msgs1", [L1["p1"]["E_PAD"], H], DT.float32)
    msgs2 = nc.dram_tensor("msgs2", [L2["p1"]["E_PAD"], H], DT.float32)
    h_own = nc.dram_tensor("h_own", [N_PAD, H], DT.float32)
    htab_own = nc.dram_tensor("htab_own", [chunk2, IN], DT.bfloat16)
    htab = nc.dram_tensor("htab", [NT2, IN], DT.bfloat16)
    pool_own = nc.dram_tensor("pool_own", [H, NGP], DT.float32)
    pool_all = nc.dram_tensor("pool_all", [8 * H, NGP], DT.float32)

    with tile.TileContext(nc) as tc:
        with (
            tc.tile_pool(name="const", bufs=1) as constp,
            tc.tile_pool(name="work", bufs=3) as work,
            tc.tile_pool(name="res", bufs=1) as resp,
        ):
            w1_t = constp.tile([IN, R, H], DT.bfloat16)
            nc.sync.dma_start(w1_t[:], w1[:, :, :])
            w2_t = constp.tile([IN, 2 * R, H], DT.bfloat16)
            nc.sync.dma_start(w2_t[:], w2[:, :, :])
            wr1_t = constp.tile([IN, H], DT.bfloat16)
            nc.sync.dma_start(wr1_t[:], wr1[:, :])
            wr2_t = constp.tile([H, H], DT.bfloat16)
            nc.sync.dma_start(wr2_t[:], wr2[:, :])
            b1_t = constp.tile([H, 1], DT.float32)
            nc.sync.dma_start(b1_t[:], b1c[:, :])
            b2_t = constp.tile([H, 1], DT.float32)
            nc.sync.dma_start(b2_t[:], b2c[:, :])
            iota128 = constp.tile([128, 512], DT.float32)
            nc.gpsimd.iota(iota128[:], [[1, 512]], channel_multiplier=0, allow_small_or_imprecise_dtypes=True)
            iotap = constp.tile([64, 1], DT.float32)
            nc.gpsimd.iota(iotap[:], [[1, 1]], channel_multiplier=1, allow_small_or_imprecise_dtypes=True)
            ident = constp.tile([64, 64], DT.bfloat16)
            nc.vector.tensor_scalar(ident[:], iota128[0:64, 0:64],
                                    iotap[:], None, AluOpType.is_equal)

            acc = resp.tile([H, N_PAD], DT.bfloat16)
            hT = resp.tile([H, N_PAD], DT.bfloat16)

            def run_layer(L, gidx_p, tabl, csize, wstack, p2g_p, seg_p, sc_p,
                          msgs_buf):
                p1, p2 = L["p1"], L["p2"]
                E_PAD, ncalls = p1["E_PAD"], p1["ncalls"]
                gtype, chunk_call0 = p1["gtype"], p1["chunk_call0"]
                # ---------- phase 1 ----------
                with (
                    tc.tile_pool(name="p1b", bufs=2) as p1b,
                    tc.tile_pool(name="p1ps", bufs=2, space="PSUM") as p1ps,
                ):
                    for call in range(min(ncalls, MAXCALL)):
                        ch = int(np.searchsorted(chunk_call0, call,
                                                 side="right") - 1)
                        t0 = call * TOK * CALL_G
                        gidx_t = p1b.tile([128, TOK * CALL_G // 16], DT.int16,
                                          tag="gidx")
                        nc.sync.dma_start(
                            gidx_t[:],
                            gidx_p[:, t0 // 16:(t0 + TOK * CALL_G) // 16])
                        xgt = p1b.tile([128, 1, TOK * CALL_G], DT.bfloat16,
                                       tag="xgt")
                        if SUB & 1:
                            nc.gpsimd.dma_gather(
                                xgt[:], tabl[ch * csize:(ch + 1) * csize, :],
                                gidx_t[:], TOK * CALL_G, TOK * CALL_G, IN,
                                transpose=True, single_packet=False)
                        else:
                            nc.gpsimd.memset(xgt[:], 0.0)
                        ps = p1ps.tile([128, CALL_G, H], DT.float32,
                                       tag="p1ps")
                        if SUB & 2:
                            for g in range(CALL_G):
                                cc = int(gtype[call * CALL_G + g])
                                nc.tensor.matmul(
                                    ps[:, g, :],
                                    xgt[:, 0, g * TOK:(g + 1) * TOK],
                                    wstack[:, cc, :], start=True, stop=True)
                        mt = p1b.tile([128, CALL_G, H], DT.float32, tag="p1m")
                        if SUB & 4:
                            nc.scalar.activation(
                                mt[:], ps[:],
                                mybir.ActivationFunctionType.Copy)
                        else:
                            nc.gpsimd.memset(mt[:], 0.0)
                        if SUB & 8:
                            nc.sync.dma_start(
                                msgs_buf[t0:t0 + TOK * CALL_G, :].rearrange(
                                    "(g p) h -> p g h", p=128), mt[:])
                # ---------- phase 2 ----------
                if STAGE < 2 + (0 if msgs_buf is msgs1 else 4):
                    return
                nrange, nslice = p2["nrange"], p2["nslice"]
                WIND, nwin = p2["WIND"], p2["nwin"]
                cnts, off = p2["cnts"], p2["off"]
                with (
                    tc.tile_pool(name="p2b", bufs=2) as p2b,
                    tc.tile_pool(name="p2ps", bufs=2, space="PSUM") as p2ps,
                ):
                    for rg in range(nrange):
                        nd = min(RANGE_D, N_PAD - rg * RANGE_D)
                        ps2 = p2ps.tile([H, RANGE_D], DT.float32, tag="p2ps")
                        for sl in range(nslice):
                            ntok = int(cnts[rg, sl].sum())
                            if ntok == 0:
                                continue
                            o = int(off[(rg * nslice + sl) * nwin])
                            g2_t = p2b.tile([128, ntok // 16], DT.int16,
                                            tag="g2")
                            nc.sync.dma_start(
                                g2_t[:], p2g_p[:, o // 16:(o + ntok) // 16])
                            sbase = sl * SLICE
                            ssize = min(SLICE, E_PAD - sbase)
                            mg = p2b.tile([128, ntok // 128, H], DT.float32,
                                          tag="mg")
                            for a in range(0, ntok, GMAX):
                                nb = min(GMAX, ntok - a)
                                nc.gpsimd.dma_gather(
                                    mg[:, a // 128:(a + nb) // 128, :],
                                    msgs_buf[sbase:sbase + ssize, :],
                                    g2_t[:, a // 16:(a + nb) // 16],
                                    nb, nb, H, single_packet=False)
                            sc_t = p2b.tile([128, ntok // 128], DT.float32,
                                            tag="scs")
                            nc.sync.dma_start(
                                sc_t[:], sc_p[:, o // 128:(o + ntok) // 128])
                            mgb = p2b.tile([128, ntok // 128, H],
                                           DT.bfloat16, tag="mgb")
                            nc.vector.tensor_tensor(
                                mgb[:], mg[:],
                                sc_t[:, :, None].broadcast_to(
                                    [128, ntok // 128, H]),
                                AluOpType.mult)
                            sg_t = p2b.tile([128, ntok // 128], DT.float32,
                                            tag="sgs")
                            nc.sync.dma_start(
                                sg_t[:], seg_p[:, o // 128:(o + ntok) // 128])
                            ind = p2b.tile([128, ntok // 128, WIND],
                                           DT.bfloat16, tag="ind")
                            nc.vector.tensor_tensor(
                                ind[:],
                                iota128[:, None, 0:WIND].broadcast_to(
                                    [128, ntok // 128, WIND]),
                                sg_t[:, :, None].broadcast_to(
                                    [128, ntok // 128, WIND]),
                                AluOpType.is_equal)
                            gg = 0
                            for w in range(nwin):
                                ngr = int(cnts[rg, sl, w]) // TOK
                                col0 = w * WIND
                                for k in range(ngr):
                                    st = (sl == 0 and k == 0
                                          and (w * WIND) % 512 == 0)
                                    nc.tensor.matmul(
                                        ps2[:, col0:col0 + WIND],
                                        mgb[:, gg, :],
                                        ind[:, gg, :],
                                        start=st, stop=False,
                                        skip_group_check=True)
                                    gg += 1
                        oc = p2b.tile([H, RANGE_D], DT.bfloat16, tag="oc")
                        nc.vector.tensor_copy(oc[:], ps2[:])
                        nc.vector.tensor_copy(
                            acc[:, rg * RANGE_D:rg * RANGE_D + nd],
                            oc[:, :nd])

            def epilogue(layer, wr_t, b_t, pool_ps):
                ntile = N_PAD // H
                with (
                    tc.tile_pool(name="epb", bufs=3) as epb,
                    tc.tile_pool(name="epps", bufs=2, space="PSUM") as epps,
                ):
                    for t in range(ntile):
                        rt = epps.tile([H, H], DT.float32, tag="root")
                        if layer == 1:
                            xtt = epb.tile([IN, H], DT.bfloat16, tag="xtt")
                            nc.sync.dma_start(xtt[:],
                                              xT[:, t * H:(t + 1) * H])
                            nc.tensor.matmul(rt[:], wr_t[:], xtt[:],
                                             start=True, stop=True)
                        else:
                            nc.tensor.matmul(rt[:], wr_t[:],
                                             hT[:, t * H:(t + 1) * H],
                                             start=True, stop=True)
                        af = epb.tile([H, H], DT.float32, tag="af")
                        nc.vector.tensor_copy(af[:],
                                              acc[:, t * H:(t + 1) * H])
                        hh = epb.tile([H, H], DT.float32, tag="hh")
                        nc.vector.tensor_tensor(hh[:], rt[:], af[:],
                                                AluOpType.add)
                        hrT = epb.tile([H, H], DT.bfloat16, tag="hrT")
                        nc.scalar.activation(hrT[:], hh[:],
                                             mybir.ActivationFunctionType.Relu,
                                             bias=b_t[:])
                        if layer == 1:
                            nc.vector.tensor_copy(hT[:, t * H:(t + 1) * H],
                                                  hrT[:])
                            # node-major rows for the gather table
                            tp = epps.tile([H, H], DT.bfloat16, tag="tp")
                            nc.tensor.transpose(tp[:], hrT[:], ident[:])
                            hrow = epb.tile([H, H], DT.float32, tag="hrow")
                            nc.vector.tensor_copy(hrow[:], tp[:])
                            nc.sync.dma_start(h_own[t * H:(t + 1) * H, :],
                                              hrow[:])
                        else:
                            # pool: out [h, g] += h2_tile(node-major) @ P
                            tp = epps.tile([H, H], DT.bfloat16, tag="tp")
                            nc.tensor.transpose(tp[:], hrT[:], ident[:])
                            h2n = epb.tile([H, H], DT.bfloat16, tag="h2n")
                            nc.vector.tensor_copy(h2n[:], tp[:])
                            pm = epb.tile([H, NG], DT.bfloat16, tag="pm")
                            nc.sync.dma_start(pm[:], pmat[t, :, :])
                            nc.tensor.matmul(pool_ps[:], h2n[:], pm[:],
                                             start=(t == 0),
                                             stop=(t == ntile - 1),
                                             skip_group_check=True)

            def dump_done(src_ap, cast=False):
                d = work.tile([128, 128], DT.float32, tag="dbgt")
                nc.gpsimd.memset(d[:], 0.0)
                if src_ap is not None:
                    if cast:
                        nc.vector.tensor_copy(d[:src_ap.shape[0], :src_ap.shape[-1]], src_ap)
                    else:
                        nc.sync.dma_start(d[:src_ap.shape[0], :src_ap.shape[-1]], src_ap)
                nc.sync.dma_start(dbg[:, :], d[:])
                o = work.tile([1, NG], DT.float32, tag="ot0")
                nc.gpsimd.memset(o[:], 0.0)
                nc.sync.dma_start(out[:, :], o[:])

            if STAGE >= 1:
                run_layer(L1, gi1, xtab, CHUNK, w1_t, p2g1, seg1, sc1, msgs1)
            if STAGE == 0:
                dump_done(None)
            elif STAGE == 1:
                dump_done(msgs1[0:128, :])
            elif STAGE == 2:
                dump_done(acc[:, 0:128], cast=True)
            if STAGE <= 2:
                pass
            else:
                epilogue(1, wr1_t, b1_t, None)
            if STAGE == 3:
                dump_done(hT[:64, 0:128], cast=True)
            if STAGE > 3:
                nc.gpsimd.dma_start(
                    out=htab_own[:, :],
                    in_=h_own[:, :].rearrange("(a b) h -> a (b h)", b=2))
                nc.gpsimd.collective_compute(
                    "AllGather", mybir.AluOpType.bypass,
                    replica_groups=[[0, 1, 2, 3], [4, 5, 6, 7]],
                    ins=[htab_own[:, :]], outs=[htab[:, :]])
            if STAGE == 4:
                dump_done(None)
            if STAGE > 4:
                run_layer(L2, gi2, htab, chunk2, w2_t, p2g2, seg2, sc2, msgs2)
            if STAGE == 5:
                dump_done(None)
            elif STAGE == 6:
                dump_done(acc[:, 0:128], cast=True)
            if STAGE < 7:
                ctx_skip = True
            else:
                ctx_skip = False
            with tc.tile_pool(name="plps", bufs=1, space="PSUM") as plps:
              if not ctx_skip:
                pool_ps = plps.tile([H, NG], DT.float32)
                epilogue(2, wr2_t, b2_t, pool_ps)
                with (
                    tc.tile_pool(name="fin", bufs=1) as fin,
                    tc.tile_pool(name="fps", bufs=1, space="PSUM") as fps,
                ):
                    pool_t = fin.tile([H, NGP], DT.float32)
                    nc.gpsimd.memset(pool_t[:], 0.0)
                    nc.vector.tensor_copy(pool_t[:, :NG], pool_ps[:])
                    nc.sync.dma_start(pool_own[:, :], pool_t[:])
                    nc.gpsimd.collective_compute(
                        "AllGather", mybir.AluOpType.bypass,
                        replica_groups=[[0, 1, 2, 3, 4, 5, 6, 7]],
                        ins=[pool_own[:, :]], outs=[pool_all[:, :]])
                    zsel_t = fin.tile([128, 8], DT.int16)
                    nc.sync.dma_start(zsel_t[:], zsel[:, :])
                    zt3 = fin.tile([128, 1, NGP], DT.float32)
                    nc.gpsimd.dma_gather(zt3[:], pool_all[:, :], zsel_t[:],
                                         128, 128, NGP)
                    zt = zt3[:, 0, :NG]
                    # depth normalization (global stats, own slice)
                    dep = fin.tile([1, NB], DT.float32)
                    nc.sync.dma_start(dep[:], depth[:, :])
                    dpo = fin.tile([1, NG], DT.float32)
                    nc.sync.dma_start(dpo[:], dep_own[:, :])
                    mean = fin.tile([1, 1], DT.float32)
                    nc.vector.tensor_reduce(mean[:], dep[:],
                                            mybir.AxisListType.X,
                                            AluOpType.add)
                    nc.vector.tensor_scalar(mean[:], mean[:], 1.0 / NB, None,
                                            AluOpType.mult)
                    cen = fin.tile([1, NB], DT.float32)
                    nc.vector.tensor_scalar(cen[:], dep[:], mean[:, 0:1],
                                            None, AluOpType.subtract)
                    sq = fin.tile([1, NB], DT.float32)
                    nc.vector.tensor_tensor(sq[:], cen[:], cen[:],
                                            AluOpType.mult)
                    var = fin.tile([1, 1], DT.float32)
                    nc.vector.tensor_reduce(var[:], sq[:],
                                            mybir.AxisListType.X,
                                            AluOpType.add)
                    nc.vector.tensor_scalar(var[:], var[:], 1.0 / NB, None,
                                            AluOpType.mult)
                    std = fin.tile([1, 1], DT.float32)
                    nc.scalar.activation(std[:], var[:],
                                         mybir.ActivationFunctionType.Sqrt)
                    nc.vector.tensor_scalar(std[:], std[:], 1e-6, None,
                                            AluOpType.add)
                    istd = fin.tile([1, 1], DT.float32)
                    nc.vector.reciprocal(istd[:], std[:])
                    dno = fin.tile([1, NG], DT.float32)
                    nc.vector.tensor_scalar(dno[:], dpo[:], mean[:, 0:1],
                                            None, AluOpType.subtract)
                    nc.vector.tensor_scalar(dno[:], dno[:], istd[:, 0:1],
                                            None, AluOpType.mult)
                    # regressor
                    rw1_t = fin.tile([2 * H, H], DT.float32)
                    nc.sync.dma_start(rw1_t[:], rw1[0:2 * H, :])
                    rw1r = fin.tile([1, H], DT.float32)
                    nc.sync.dma_start(rw1r[:], rw1[2 * H:2 * H + 1, :])
                    rb1_t = fin.tile([H, 1], DT.float32)
                    nc.sync.dma_start(rb1_t[:], rb1[:, :])
                    rw2_t = fin.tile([H, 1], DT.float32)
                    nc.sync.dma_start(rw2_t[:], rw2[:, :])
                    rb2_t = fin.tile([1, 1], DT.float32)
                    nc.sync.dma_start(rb2_t[:], rb2c[:, :])
                    r1 = fps.tile([H, NG], DT.float32)
                    nc.tensor.matmul(r1[:], rw1_t[:], zt[:],
                                     start=True, stop=False,
                                     skip_group_check=True)
                    nc.tensor.matmul(r1[:], rw1r[:], dno[:],
                                     start=False, stop=True,
                                     skip_group_check=True)
                    r1s = fin.tile([H, NG], DT.float32)
                    nc.scalar.activation(r1s[:], r1[:],
                                         mybir.ActivationFunctionType.Relu,
                                         bias=rb1_t[:])
                    r2 = fps.tile([1, NG], DT.float32)
                    nc.tensor.matmul(r2[:], rw2_t[:], r1s[:], start=True,
                                     stop=True)
                    ot = fin.tile([1, NG], DT.float32)
                    nc.vector.tensor_scalar(ot[:], r2[:], rb2_t[:, 0:1],
                                            None, AluOpType.add)
                    nc.sync.dma_start(out[:, :], ot[:])

    nc.compile()
    return nc


# ---------------------------------------------------------------------------
# entry point
# ---------------------------------------------------------------------------

def kernel(**inputs):
    inp = {k: np.asarray(v) for k, v in inputs.items()}
    N = inp["state_x"].shape[0]
    NB = inp["depth"].shape[0]
    NG = NB // NCORES_ENC

    st = plan_encoder(inp["state_x"], inp["state_edge_index"],
                      inp["state_edge_type"], inp["state_batch"], NB)
    go = plan_encoder(inp["goal_x"], inp["goal_edge_index"],
                      inp["goal_edge_type"], inp["goal_batch"], NB)
    cores = st + go
    N_PAD = rup(max(c["n_own"] for c in cores), RANGE_D)
    NT1 = rup(N, CHUNK)
    nchunks1 = NT1 // CHUNK
    chunk2 = N_PAD // 2
    NT2 = NCORES_ENC * chunk2

    for c in cores:
        c["tsrc"] = c["src"]
        c["tpar"] = np.zeros(len(c["src"]), np.int64)
    p1_1 = build_phase1(cores, CHUNK, nchunks1, 1)
    p2_1 = build_phase2(p1_1, N_PAD)

    for enc, grp in ((0, st), (1, go)):
        nsb = np.array([c["ns"] for c in grp] + [N], np.int64)
        for c in grp:
            owner = np.searchsorted(nsb, c["src"], side="right") - 1
            loc = c["src"] - nsb[owner]
            c["tsrc"] = owner * chunk2 + loc // 2
            c["tpar"] = loc % 2
    p1_2 = build_phase1(cores, chunk2, NCORES_ENC, 2)
    p2_2 = build_phase2(p1_2, N_PAD)

    L1 = dict(p1=p1_1, p2=p2_1)
    L2 = dict(p1=p1_2, p2=p2_2)
    cfg = dict(N_PAD=N_PAD, NT1=NT1, NT2=NT2, L1=L1, L2=L2, NG=NG, NB=NB)
    nc = build_kernel(cfg)

    in_maps = []
    for ci in range(8):
        enc = ci // NCORES_ENC
        c = cores[ci]
        pfx = "s" if enc == 0 else "g"
        x = inp["state_x"] if enc == 0 else inp["goal_x"]
        batch = inp["state_batch"] if enc == 0 else inp["goal_batch"]
        xt = np.zeros((NT1, IN), BF16)
        xt[:N] = x.astype(BF16)
        xTv = np.zeros((IN, N_PAD), BF16)
        xTv[:, :c["n_own"]] = x[c["ns"]:c["ne"]].T.astype(BF16)
        W1 = np.ascontiguousarray(
            inp[pfx + "W1"].astype(BF16).transpose(1, 0, 2))
        W2s = np.zeros((IN, 2 * R, H), BF16)
        W2s[:H, :R] = inp[pfx + "W2"].astype(BF16).transpose(1, 0, 2)
        W2s[H:2 * H, R:] = inp[pfx + "W2"].astype(BF16).transpose(1, 0, 2)
        g0 = (ci % NCORES_ENC) * NG
        bown = batch[c["ns"]:c["ne"]] - g0
        gcnt = np.maximum(
            np.bincount(bown, minlength=NG).astype(np.float32), 1.0)
        pm = np.zeros((N_PAD // H, H, NG), BF16)
        nidx = np.arange(c["n_own"])
        pm[nidx // H, nidx % H, bown] = (1.0 / gcnt[bown]).astype(BF16)
        im = dict(
            xtab=xt, xT=xTv,
            w1=W1, wr1=inp[pfx + "Root1"].astype(BF16),
            w2=W2s, wr2=inp[pfx + "Root2"].astype(BF16),
            b1c=inp[pfx + "B1"].astype(np.float32)[:, None],
            b2c=inp[pfx + "B2"].astype(np.float32)[:, None],
            gi1=wrap16(p1_1["per_core"][ci]["gidx"]),
            gi2=wrap16(p1_2["per_core"][ci]["gidx"]),
            p2g1=wrap16(p2_1["per_core"][ci]["g2"]),
            p2g2=wrap16(p2_2["per_core"][ci]["g2"]),
            seg1=wrap128(p2_1["per_core"][ci]["seg"]),
            seg2=wrap128(p2_2["per_core"][ci]["seg"]),
            sc1=wrap128(p2_1["per_core"][ci]["sc2"]),
            sc2=wrap128(p2_2["per_core"][ci]["sc2"]),
            pmat=pm,
            depth=inp["depth"].astype(np.float32)[None, :],
            dep_own=inp["depth"].astype(np.float32)[None, g0:g0 + NG],
            rw1=inp["rW1"].astype(np.float32),
            rb1=inp["rB1"].astype(np.float32)[:, None],
            rw2=inp["rW2"].astype(np.float32),
            rb2c=inp["rB2"].astype(np.float32)[None, :],
            zsel=wrap16(np.r_[(ci % NCORES_ENC) * H + np.arange(H),
                              ((ci % NCORES_ENC) + NCORES_ENC) * H
                              + np.arange(H)].astype(np.int16)),
        )
        in_maps.append(im)

    res = bass_utils.run_bass_kernel_spmd(nc, in_maps,
                                          core_ids=list(range(8)))
    global LAST_DBG, LAST_PLANS, LAST_RES
    LAST_RES = res
    LAST_DBG = [res.results[c].get("dbg") for c in range(8)]
    LAST_PLANS = (cores, L1, L2, cfg)
    outp = np.zeros(NB, np.float32)
    for ci in range(NCORES_ENC):
        outp[ci * NG:(ci + 1) * NG] = res.results[ci]["out"][0]
    return outp



# revision 7
# speedup vs baseline: 1.2042x; 1.0385x over previous
"""Distance-estimator (2x RGCN encoder + regressor) on 8 TRN2 NeuronCores.

Sharding: cores 0-3 = state encoder, 4-7 = goal encoder (SPMD, different
data). Each core owns a contiguous quarter of the graphs -> contiguous
dst-node range (batch sorted). Per core-layer:

Phase 1: edges sorted by (src-chunk, type[, parity]); dma_gather source rows
  (bf16, transposed) from the feature table; per-128-token matmul with W_type
  -> msgs [tok, 64] f32 written sequentially to an HBM buffer.
Phase 2: msgs re-gathered in (dst-range, msgs-slice) order: for each range
  of 2048 dst nodes x each 32k-row msgs slice (int16 idx), gather the
  tokens, scale by 1/cnt(dst,type), aggregate with indicator matmuls
  (lhsT = one-hot of dst-within-group built via iota/is_equal) writing
  dense node-columns of a PSUM [64, 2048] accumulator -> acc [64h, N_PAD]
  bf16 image (transposed orientation).
Epilogue (transposed): accT + Wroot^T@xT + b -> relu -> hT tiles; L1 emits
  the pair-packed bf16 h table (AllGather across the encoder's 4 cores);
  L2 pools via matmul with host P matrices; paired cores exchange pooled
  embeddings; per-core f32 regressor for its 128 graphs.
"""
import os
import sys
sys.path.insert(0, "/opt/trn_rl_repo")
STAGE = int(os.environ.get("K_STAGE", "99"))
SUB = int(os.environ.get("K_SUB", "15"))
MAXCALL = int(os.environ.get("K_MAXCALL", "100000"))
import numpy as np
import ml_dtypes

from concourse import bass, bacc, tile
from concourse import bass_utils
from concourse.bass import mybir
from concourse.alu_op_type import AluOpType

DT = mybir.dt
BF16 = ml_dtypes.bfloat16
LAST_DBG = None
LAST_PLANS = None
LAST_RES = None

IN = 128
H = 64
R = 32
TOK = 128
CALL_G = int(os.environ.get("K_CALLG", "16"))
GMAX = 4096              # max idxs per dma_gather call
RANGE_D = 2048           # dsts per phase-2 range (psum [64, 2048])
SLICE = 32512            # msgs rows per phase-2 slice (int16-safe)
CHUNK = 25088
NCORES_ENC = 4
BANK = 512               # psum bank cols (f32)


def cdiv(a, b):
    return (a + b - 1) // b


def rup(a, b):
    return cdiv(a, b) * b


def wrap16(a):
    w = a.reshape(-1, 16).T
    return np.tile(w, (8, 1)).copy()


def wrap128(a):
    return np.ascontiguousarray(a.reshape(-1, 128).T)


# ---------------------------------------------------------------------------
# host planning
# ---------------------------------------------------------------------------

def plan_encoder(x, edge_index, edge_type, batch, n_graphs):
    N = x.shape[0]
    src, dst = edge_index[0].astype(np.int64), edge_index[1].astype(np.int64)
    et = edge_type.astype(np.int64)
    gpc = n_graphs // NCORES_ENC
    bounds = [int(np.searchsorted(batch, c * gpc)) for c in range(NCORES_ENC)] + [N]
    key = dst * R + et
    uniq, inv, cnt = np.unique(key, return_inverse=True, return_counts=True)
    scale_all = (1.0 / cnt[inv]).astype(np.float32)
    cores = []
    for c in range(NCORES_ENC):
        ns, ne = bounds[c], bounds[c + 1]
        m = (dst >= ns) & (dst < ne)
        cores.append(dict(ns=ns, ne=ne, n_own=ne - ns,
                          src=src[m], dstl=(dst[m] - ns), et=et[m],
                          scale=scale_all[m]))
    return cores


def rank_within_sorted(keys_sorted):
    n = len(keys_sorted)
    if n == 0:
        return np.zeros(0, np.int64)
    first = np.r_[True, keys_sorted[1:] != keys_sorted[:-1]]
    starts = np.where(first)[0]
    sizes = np.diff(np.r_[starts, n])
    return np.arange(n) - np.repeat(starts, sizes)


def build_phase1(cores8, chunk_size, nchunks, parity_cells):
    """Common phase-1 schedule + per-core gidx streams and token dst/scale."""
    ncell = nchunks * parity_cells * R
    cells_per_chunk = parity_cells * R
    cell_of = []
    for co in cores8:
        chunk = co["tsrc"] // chunk_size
        cell_of.append((chunk * parity_cells + co["tpar"]) * R + co["et"])
    sizes = np.zeros(ncell, np.int64)
    for cell in cell_of:
        sizes = np.maximum(sizes, np.bincount(cell, minlength=ncell))
    sizes = rup(sizes, TOK)
    chunk_tok = rup(sizes.reshape(nchunks, cells_per_chunk).sum(1),
                    TOK * CALL_G)
    E_PAD = int(chunk_tok.sum())
    ncalls = E_PAD // (TOK * CALL_G)
    cell_off = np.zeros(ncell, np.int64)
    chunk_call0 = np.zeros(nchunks + 1, np.int64)
    off = 0
    for ch in range(nchunks):
        chunk_call0[ch] = off // (TOK * CALL_G)
        for cc in range(cells_per_chunk):
            i = ch * cells_per_chunk + cc
            cell_off[i] = off
            off += sizes[i]
        off = int(rup(off, TOK * CALL_G))
    chunk_call0[nchunks] = ncalls
    gtype = np.zeros(E_PAD // TOK, np.int64)
    for ch in range(nchunks):
        for cc in range(cells_per_chunk):
            i = ch * cells_per_chunk + cc
            o, s = int(cell_off[i]), int(sizes[i])
            gtype[o // TOK:(o + s) // TOK] = cc
    per_core = []
    for ci, co in enumerate(cores8):
        cell = cell_of[ci]
        order = np.argsort(cell, kind="stable")
        pos = cell_off[cell[order]] + rank_within_sorted(cell[order])
        gidx = np.zeros(E_PAD, np.int16)
        scale = np.zeros(E_PAD, np.float32)
        dstl = np.full(E_PAD, -1, np.int64)
        gidx[pos] = (co["tsrc"][order] % chunk_size).astype(np.int16)
        scale[pos] = co["scale"][order]
        dstl[pos] = co["dstl"][order]
        per_core.append(dict(gidx=gidx, scale=scale, dstl=dstl))
    return dict(E_PAD=E_PAD, ncalls=ncalls, gtype=gtype,
                chunk_call0=chunk_call0, per_core=per_core)


def build_phase2(p1, N_PAD):
    """Phase-2 schedule. Per (range, slice): per-core token lists sorted by
    dst; groups of 128 tokens; per group the distinct-dst count and psum
    col offset (= first dst - range base) must be SPMD-common -> we make the
    *group schedule* common by padding: for each (rg, sl) take the max group
    count over cores; per group, the indicator handles arbitrary per-core
    content (seg-in-group = dstl - group_first_dst, with group_first_dst a
    PER-CORE value baked into the seg stream; the psum col offset however is
    an instruction attribute!).

    To keep psum col offsets common across cores we use a fixed mapping:
    group g of (rg, sl) writes cols [0, RANGE_D) via indicator width d=128
    at col offset off_g where off_g is per-(rg,sl,g) COMMON. We achieve this
    by giving every group the FULL 128-wide indicator with seg-in-group =
    dstl - off_g - rg*RANGE_D, where off_g = common col offset. Host picks
    off_g = min over cores of (first dst in group) clamped so that all
    cores' dsts in group g fall in [off_g, off_g+128). This requires group
    contents aligned across cores -> instead we simply sort each core's
    tokens by dst and pick off_g = max(0, min(RANGE_D-128, g_quantile)):
    with tokens sorted by dst, group g's dsts lie near the g-th quantile of
    the range. Because token counts per (rg, sl) differ per core, quantiles
    differ -> we pad each (rg, sl) per-core token list to the common size
    first (pad tokens: scale 0, dst = off_g so they hit a valid col), THEN
    split into groups; off_g = max over cores of group-first-dst... still
    per-core spread. Resolution: indicator width 128 covers a 128-dst
    window; we pick off_g common = min over cores of first-dst; cores whose
    group spans beyond off_g+128 would lose tokens -> prevented by ALIGNING
    group boundaries to dst windows of 64: tokens are binned into windows of
    64 dsts; per (rg, sl, win64) the token count is padded to the common max
    (so group boundaries = window boundaries are common); groups within a
    window: indicator width 64 at col offset win*64 (common). Window token
    counts per (rg, sl, win) ~ Poisson(small) -> pad overhead high if wins
    are shredded... accept: win=64, tokens per (rg,sl,win) ~= 64*16.4/13
    ~= 81 -> padded max-over-8-cores ~= 81+4*9 ~= 117 -> rup128 -> 128:
    pad ~60%?? Too high. Use win = RANGE_D (one window per range, indicator
    128 wide can't cover 2048)... Final compromise: win = 128 dsts,
    indicator width 128, tokens per (rg, sl, win128) ~= 162 -> padded
    ~rup(162+50) = 256 vs 162: ~45% pad. Ouch but correct and simple.
    Actually choose win = 256 with TWO indicator mms per group (cols split
    at 128): tokens/win ~= 324 -> pad to ~rup(324+64,128)=512?? 30%.
    We go with win=128 for simplicity. pad measured at runtime.
    """
    E_PAD = p1["E_PAD"]
    nslice = cdiv(E_PAD, SLICE)
    nrange = cdiv(N_PAD, RANGE_D)
    WIND = 512
    nwin = RANGE_D // WIND
    # per (rg, sl, win) common padded token counts
    cnts = np.zeros((nrange, nslice, nwin), np.int64)
    toks = []
    for pc in p1["per_core"]:
        dstl = pc["dstl"]
        t = np.where(dstl >= 0)[0]
        rg = dstl[t] // RANGE_D
        sl = t // SLICE
        win = (dstl[t] % RANGE_D) // WIND
        c = np.zeros((nrange, nslice, nwin), np.int64)
        np.add.at(c, (rg, sl, win), 1)
        cnts = np.maximum(cnts, c)
        toks.append((t, rg, sl, win))
    cnts = rup(cnts, TOK)
    cnts[:, 0, :] = np.maximum(cnts[:, 0, :], TOK)
    # offsets in the phase-2 stream
    flat = cnts.reshape(-1)
    off = np.zeros(len(flat) + 1, np.int64)
    off[1:] = np.cumsum(flat)
    P2_PAD = int(off[-1])
    # static group schedule: for each (rg, sl): list of (win, ngroups)
    per_core = []
    for ci, pc in enumerate(p1["per_core"]):
        t, rg, sl, win = toks[ci]
        dstl = pc["dstl"]
        key = (rg * nslice + sl) * nwin + win
        order = np.argsort((key * (E_PAD + 1) + t), kind="stable")
        ts = t[order]
        ks = key[order]
        pos = off[ks] + rank_within_sorted(ks)
        g2 = np.zeros(P2_PAD, np.int16)
        seg = np.zeros(P2_PAD, np.float32)
        sc2 = np.zeros(P2_PAD, np.float32)
        g2[pos] = (ts % SLICE).astype(np.int16)
        seg[pos] = (dstl[ts] % WIND).astype(np.float32)
        sc2[pos] = pc["scale"][ts]
        per_core.append(dict(g2=g2, seg=seg, sc2=sc2))
    return dict(nslice=nslice, nrange=nrange, WIND=WIND, nwin=nwin,
                cnts=cnts, off=off, P2_PAD=P2_PAD, per_core=per_core)


# ---------------------------------------------------------------------------
# kernel builder
# ---------------------------------------------------------------------------

def build_kernel(cfg):
    N_PAD, NT1, NT2 = cfg["N_PAD"], cfg["NT1"], cfg["NT2"]
    L1, L2 = cfg["L1"], cfg["L2"]
    NG, NB = cfg["NG"], cfg["NB"]
    NGP = rup(max(NG, 64), 64)
    chunk2 = N_PAD // 2

    nc = bacc.Bacc(None, target_bir_lowering=False, debug=False)

    def din(name, shape, dt):
        return nc.declare_dram_parameter(name, list(shape), dt, isOutput=False)

    xtab = din("xtab", [NT1, IN], DT.bfloat16)
    xT = din("xT", [IN, N_PAD], DT.bfloat16)
    w1 = din("w1", [IN, R, H], DT.bfloat16)
    wr1 = din("wr1", [IN, H], DT.bfloat16)
    w2 = din("w2", [IN, 2 * R, H], DT.bfloat16)
    wr2 = din("wr2", [H, H], DT.bfloat16)
    b1c = din("b1c", [H, 1], DT.float32)
    b2c = din("b2c", [H, 1], DT.float32)
    gi1 = din("gi1", [128, L1["p1"]["E_PAD"] // 16], DT.int16)
    gi2 = din("gi2", [128, L2["p1"]["E_PAD"] // 16], DT.int16)
    p2g1 = din("p2g1", [128, L1["p2"]["P2_PAD"] // 16], DT.int16)
    p2g2 = din("p2g2", [128, L2["p2"]["P2_PAD"] // 16], DT.int16)
    seg1 = din("seg1", [128, L1["p2"]["P2_PAD"] // 128], DT.float32)
    seg2 = din("seg2", [128, L2["p2"]["P2_PAD"] // 128], DT.float32)
    sc1 = din("sc1", [128, L1["p2"]["P2_PAD"] // 128], DT.float32)
    sc2 = din("sc2", [128, L2["p2"]["P2_PAD"] // 128], DT.float32)
    pmat = din("pmat", [N_PAD // H, H, NG], DT.bfloat16)
    depth = din("depth", [1, NB], DT.float32)
    dep_own = din("dep_own", [1, NG], DT.float32)
    zsel = din("zsel", [128, 8], DT.int16)
    rw1 = din("rw1", [2 * H + 1, H], DT.float32)
    rb1 = din("rb1", [H, 1], DT.float32)
    rw2 = din("rw2", [H, 1], DT.float32)
    rb2c = din("rb2c", [1, 1], DT.float32)
    out = nc.declare_dram_parameter("out", [1, NG], DT.float32, isOutput=True)
    dbg = nc.declare_dram_parameter("dbg", [128, 128], DT.float32, isOutput=True)

    msgs1 = nc.dram_tensor("msgs1", [L1["p1"]["E_PAD"], H], DT.float32)
    msgs2 = nc.dram_tensor("msgs2", [L2["p1"]["E_PAD"], H], DT.float32)
    h_own = nc.dram_tensor("h_own", [N_PAD, H], DT.float32)
    htab_own = nc.dram_tensor("htab_own", [chunk2, IN], DT.bfloat16)
    htab = nc.dram_tensor("htab", [NT2, IN], DT.bfloat16)
    pool_own = nc.dram_tensor("pool_own", [H, NGP], DT.float32)
    pool_all = nc.dram_tensor("pool_all", [8 * H, NGP], DT.float32)

    with tile.TileContext(nc) as tc:
        with (
            tc.tile_pool(name="const", bufs=1) as constp,
            tc.tile_pool(name="work", bufs=3) as work,
            tc.tile_pool(name="res", bufs=1) as resp,
        ):
            w1_t = constp.tile([IN, R, H], DT.bfloat16)
            nc.sync.dma_start(w1_t[:], w1[:, :, :])
            w2_t = constp.tile([IN, 2 * R, H], DT.bfloat16)
            nc.sync.dma_start(w2_t[:], w2[:, :, :])
            wr1_t = constp.tile([IN, H], DT.bfloat16)
            nc.sync.dma_start(wr1_t[:], wr1[:, :])
            wr2_t = constp.tile([H, H], DT.bfloat16)
            nc.sync.dma_start(wr2_t[:], wr2[:, :])
            b1_t = constp.tile([H, 1], DT.float32)
            nc.sync.dma_start(b1_t[:], b1c[:, :])
            b2_t = constp.tile([H, 1], DT.float32)
            nc.sync.dma_start(b2_t[:], b2c[:, :])
            iota128 = constp.tile([128, 512], DT.float32)
            nc.gpsimd.iota(iota128[:], [[1, 512]], channel_multiplier=0, allow_small_or_imprecise_dtypes=True)
            iotap = constp.tile([64, 1], DT.float32)
            nc.gpsimd.iota(iotap[:], [[1, 1]], channel_multiplier=1, allow_small_or_imprecise_dtypes=True)
            ident = constp.tile([64, 64], DT.bfloat16)
            nc.vector.tensor_scalar(ident[:], iota128[0:64, 0:64],
                                    iotap[:], None, AluOpType.is_equal)

            acc = resp.tile([H, N_PAD], DT.bfloat16)
            hT = resp.tile([H, N_PAD], DT.bfloat16)

            def run_layer(L, gidx_p, tabl, csize, wstack, p2g_p, seg_p, sc_p,
                          msgs_buf):
                p1, p2 = L["p1"], L["p2"]
                E_PAD, ncalls = p1["E_PAD"], p1["ncalls"]
                gtype, chunk_call0 = p1["gtype"], p1["chunk_call0"]
                # ---------- phase 1 ----------
                with (
                    tc.tile_pool(name="p1b", bufs=2) as p1b,
                    tc.tile_pool(name="p1ps", bufs=2, space="PSUM") as p1ps,
                ):
                    for call in range(min(ncalls, MAXCALL)):
                        ch = int(np.searchsorted(chunk_call0, call,
                                                 side="right") - 1)
                        t0 = call * TOK * CALL_G
                        gidx_t = p1b.tile([128, TOK * CALL_G // 16], DT.int16,
                                          tag="gidx")
                        nc.sync.dma_start(
                            gidx_t[:],
                            gidx_p[:, t0 // 16:(t0 + TOK * CALL_G) // 16])
                        xgt = p1b.tile([128, 1, TOK * CALL_G], DT.bfloat16,
                                       tag="xgt")
                        if SUB & 1:
                            nc.gpsimd.dma_gather(
                                xgt[:], tabl[ch * csize:(ch + 1) * csize, :],
                                gidx_t[:], TOK * CALL_G, TOK * CALL_G, IN,
                                transpose=True, single_packet=False)
                        else:
                            nc.gpsimd.memset(xgt[:], 0.0)
                        ps = p1ps.tile([128, CALL_G, H], DT.float32,
                                       tag="p1ps")
                        if SUB & 2:
                            for g in range(CALL_G):
                                cc = int(gtype[call * CALL_G + g])
                                nc.tensor.matmul(
                                    ps[:, g, :],
                                    xgt[:, 0, g * TOK:(g + 1) * TOK],
                                    wstack[:, cc, :], start=True, stop=True)
                        mt = p1b.tile([128, CALL_G, H], DT.float32, tag="p1m")
                        if SUB & 4:
                            nc.scalar.activation(
                                mt[:], ps[:],
                                mybir.ActivationFunctionType.Copy)
                        else:
                            nc.gpsimd.memset(mt[:], 0.0)
                        if SUB & 8:
                            nc.sync.dma_start(
                                msgs_buf[t0:t0 + TOK * CALL_G, :].rearrange(
                                    "(g p) h -> p g h", p=128), mt[:])
                # ---------- phase 2 ----------
                if STAGE < 2 + (0 if msgs_buf is msgs1 else 4):
                    return
                nrange, nslice = p2["nrange"], p2["nslice"]
                WIND, nwin = p2["WIND"], p2["nwin"]
                cnts, off = p2["cnts"], p2["off"]
                with (
                    tc.tile_pool(name="p2b", bufs=2) as p2b,
                    tc.tile_pool(name="p2ps", bufs=2, space="PSUM") as p2ps,
                ):
                    for rg in range(nrange):
                        nd = min(RANGE_D, N_PAD - rg * RANGE_D)
                        ps2 = p2ps.tile([H, RANGE_D], DT.float32, tag="p2ps")
                        for sl in range(nslice):
                            ntok = int(cnts[rg, sl].sum())
                            if ntok == 0:
                                continue
                            o = int(off[(rg * nslice + sl) * nwin])
                            g2_t = p2b.tile([128, ntok // 16], DT.int16,
                                            tag="g2")
                            nc.sync.dma_start(
                                g2_t[:], p2g_p[:, o // 16:(o + ntok) // 16])
                            sbase = sl * SLICE
                            ssize = min(SLICE, E_PAD - sbase)
                            mg = p2b.tile([128, ntok // 128, H], DT.float32,
                                          tag="mg")
                            for a in range(0, ntok, GMAX):
                                nb = min(GMAX, ntok - a)
                                nc.gpsimd.dma_gather(
                                    mg[:, a // 128:(a + nb) // 128, :],
                                    msgs_buf[sbase:sbase + ssize, :],
                                    g2_t[:, a // 16:(a + nb) // 16],
                                    nb, nb, H, single_packet=False)
                            sc_t = p2b.tile([128, ntok // 128], DT.float32,
                                            tag="scs")
                            nc.sync.dma_start(
                                sc_t[:], sc_p[:, o // 128:(o + ntok) // 128])
                            mgb = p2b.tile([128, ntok // 128, H],
                                           DT.bfloat16, tag="mgb")
                            nc.vector.tensor_tensor(
                                mgb[:], mg[:],
                                sc_t[:, :, None].broadcast_to(
                                    [128, ntok // 128, H]),
                                AluOpType.mult)
                            sg_t = p2b.tile([128, ntok // 128], DT.float32,
                                            tag="sgs")
                            nc.sync.dma_start(
                                sg_t[:], seg_p[:, o // 128:(o + ntok) // 128])
                            ind = p2b.tile([128, ntok // 128, WIND],
                                           DT.bfloat16, tag="ind")
                            nc.vector.tensor_tensor(
                                ind[:],
                                iota128[:, None, 0:WIND].broadcast_to(
                                    [128, ntok // 128, WIND]),
                                sg_t[:, :, None].broadcast_to(
                                    [128, ntok // 128, WIND]),
                                AluOpType.is_equal)
                            gg = 0
                            for w in range(nwin):
                                ngr = int(cnts[rg, sl, w]) // TOK
                                col0 = w * WIND
                                for k in range(ngr):
                                    st = (sl == 0 and k == 0
                                          and (w * WIND) % 512 == 0)
                                    nc.tensor.matmul(
                                        ps2[:, col0:col0 + WIND],
                                        mgb[:, gg, :],
                                        ind[:, gg, :],
                                        start=st, stop=False,
                                        skip_group_check=True)
                                    gg += 1
                        oc = p2b.tile([H, RANGE_D], DT.bfloat16, tag="oc")
                        nc.vector.tensor_copy(oc[:], ps2[:])
                        nc.vector.tensor_copy(
                            acc[:, rg * RANGE_D:rg * RANGE_D + nd],
                            oc[:, :nd])

            def epilogue(layer, wr_t, b_t, pool_ps):
                ntile = N_PAD // H
                with (
                    tc.tile_pool(name="epb", bufs=3) as epb,
                    tc.tile_pool(name="epps", bufs=2, space="PSUM") as epps,
                ):
                    for t in range(ntile):
                        rt = epps.tile([H, H], DT.float32, tag="root")
                        if layer == 1:
                            xtt = epb.tile([IN, H], DT.bfloat16, tag="xtt")
                            nc.sync.dma_start(xtt[:],
                                              xT[:, t * H:(t + 1) * H])
                            nc.tensor.matmul(rt[:], wr_t[:], xtt[:],
                                             start=True, stop=True)
                        else:
                            nc.tensor.matmul(rt[:], wr_t[:],
                                             hT[:, t * H:(t + 1) * H],
                                             start=True, stop=True)
                        af = epb.tile([H, H], DT.float32, tag="af")
                        nc.vector.tensor_copy(af[:],
                                              acc[:, t * H:(t + 1) * H])
                        hh = epb.tile([H, H], DT.float32, tag="hh")
                        nc.vector.tensor_tensor(hh[:], rt[:], af[:],
                                                AluOpType.add)
                        hrT = epb.tile([H, H], DT.bfloat16, tag="hrT")
                        nc.scalar.activation(hrT[:], hh[:],
                                             mybir.ActivationFunctionType.Relu,
                                             bias=b_t[:])
                        if layer == 1:
                            nc.vector.tensor_copy(hT[:, t * H:(t + 1) * H],
                                                  hrT[:])
                            # node-major rows for the gather table
                            tp = epps.tile([H, H], DT.bfloat16, tag="tp")
                            nc.tensor.transpose(tp[:], hrT[:], ident[:])
                            hrow = epb.tile([H, H], DT.float32, tag="hrow")
                            nc.vector.tensor_copy(hrow[:], tp[:])
                            nc.sync.dma_start(h_own[t * H:(t + 1) * H, :],
                                              hrow[:])
                        else:
                            # pool: out [h, g] += h2_tile(node-major) @ P
                            tp = epps.tile([H, H], DT.bfloat16, tag="tp")
                            nc.tensor.transpose(tp[:], hrT[:], ident[:])
                            h2n = epb.tile([H, H], DT.bfloat16, tag="h2n")
                            nc.vector.tensor_copy(h2n[:], tp[:])
                            pm = epb.tile([H, NG], DT.bfloat16, tag="pm")
                            nc.sync.dma_start(pm[:], pmat[t, :, :])
                            nc.tensor.matmul(pool_ps[:], h2n[:], pm[:],
                                             start=(t == 0),
                                             stop=(t == ntile - 1),
                                             skip_group_check=True)

            def dump_done(src_ap, cast=False):
                d = work.tile([128, 128], DT.float32, tag="dbgt")
                nc.gpsimd.memset(d[:], 0.0)
                if src_ap is not None:
                    if cast:
                        nc.vector.tensor_copy(d[:src_ap.shape[0], :src_ap.shape[-1]], src_ap)
                    else:
                        nc.sync.dma_start(d[:src_ap.shape[0], :src_ap.shape[-1]], src_ap)
                nc.sync.dma_start(dbg[:, :], d[:])
                o = work.tile([1, NG], DT.float32, tag="ot0")
                nc.gpsimd.memset(o[:], 0.0)
                nc.sync.dma_start(out[:, :], o[:])

            if STAGE >= 1:
                run_layer(L1, gi1, xtab, CHUNK, w1_t, p2g1, seg1, sc1, msgs1)
            if STAGE == 0:
                dump_done(None)
            elif STAGE == 1:
                dump_done(msgs1[0:128, :])
            elif STAGE == 2:
                dump_done(acc[:, 0:128], cast=True)
            if STAGE <= 2:
                pass
            else:
                epilogue(1, wr1_t, b1_t, None)
            if STAGE == 3:
                dump_done(hT[:64, 0:128], cast=True)
            if STAGE > 3:
                nc.gpsimd.dma_start(
                    out=htab_own[:, :],
                    in_=h_own[:, :].rearrange("(a b) h -> a (b h)", b=2))
                nc.gpsimd.collective_compute(
                    "AllGather", mybir.AluOpType.bypass,
                    replica_groups=[[0, 1, 2, 3], [4, 5, 6, 7]],
                    ins=[htab_own[:, :]], outs=[htab[:, :]])
            if STAGE == 4:
                dump_done(None)
            if STAGE > 4:
                run_layer(L2, gi2, htab, chunk2, w2_t, p2g2, seg2, sc2, msgs2)
            if STAGE == 5:
                dump_done(None)
            elif STAGE == 6:
                dump_done(acc[:, 0:128], cast=True)
            if STAGE < 7:
                ctx_skip = True
            else:
                ctx_skip = False
            with tc.tile_pool(name="plps", bufs=1, space="PSUM") as plps:
              if not ctx_skip:
                pool_ps = plps.tile([H, NG], DT.float32)
                epilogue(2, wr2_t, b2_t, pool_ps)
                with (
                    tc.tile_pool(name="fin", bufs=1) as fin,
                    tc.tile_pool(name="fps", bufs=1, space="PSUM") as fps,
                ):
                    pool_t = fin.tile([H, NGP], DT.float32)
                    nc.gpsimd.memset(pool_t[:], 0.0)
                    nc.vector.tensor_copy(pool_t[:, :NG], pool_ps[:])
                    nc.sync.dma_start(pool_own[:, :], pool_t[:])
                    nc.gpsimd.collective_compute(
                        "AllGather", mybir.AluOpType.bypass,
                        replica_groups=[[0, 1, 2, 3, 4, 5, 6, 7]],
                        ins=[pool_own[:, :]], outs=[pool_all[:, :]])
                    zsel_t = fin.tile([128, 8], DT.int16)
                    nc.sync.dma_start(zsel_t[:], zsel[:, :])
                    zt3 = fin.tile([128, 1, NGP], DT.float32)
                    nc.gpsimd.dma_gather(zt3[:], pool_all[:, :], zsel_t[:],
                                         128, 128, NGP)
                    zt = zt3[:, 0, :NG]
                    # depth normalization (global stats, own slice)
                    dep = fin.tile([1, NB], DT.float32)
                    nc.sync.dma_start(dep[:], depth[:, :])
                    dpo = fin.tile([1, NG], DT.float32)
                    nc.sync.dma_start(dpo[:], dep_own[:, :])
                    mean = fin.tile([1, 1], DT.float32)
                    nc.vector.tensor_reduce(mean[:], dep[:],
                                            mybir.AxisListType.X,
                                            AluOpType.add)
                    nc.vector.tensor_scalar(mean[:], mean[:], 1.0 / NB, None,
                                            AluOpType.mult)
                    cen = fin.tile([1, NB], DT.float32)
                    nc.vector.tensor_scalar(cen[:], dep[:], mean[:, 0:1],
                                            None, AluOpType.subtract)
                    sq = fin.tile([1, NB], DT.float32)
                    nc.vector.tensor_tensor(sq[:], cen[:], cen[:],
                                            AluOpType.mult)
                    var = fin.tile([1, 1], DT.float32)
                    nc.vector.tensor_reduce(var[:], sq[:],
                                            mybir.AxisListType.X,
                                            AluOpType.add)
                    nc.vector.tensor_scalar(var[:], var[:], 1.0 / NB, None,
                                            AluOpType.mult)
                    std = fin.tile([1, 1], DT.float32)
                    nc.scalar.activation(std[:], var[:],
                                         mybir.ActivationFunctionType.Sqrt)
                    nc.vector.tensor_scalar(std[:], std[:], 1e-6, None,
                                            AluOpType.add)
                    istd = fin.tile([1, 1], DT.float32)
                    nc.vector.reciprocal(istd[:], std[:])
                    dno = fin.tile([1, NG], DT.float32)
                    nc.vector.tensor_scalar(dno[:], dpo[:], mean[:, 0:1],
                                            None, AluOpType.subtract)
                    nc.vector.tensor_scalar(dno[:], dno[:], istd[:, 0:1],
                                            None, AluOpType.mult)
                    # regressor
                    rw1_t = fin.tile([2 * H, H], DT.float32)
                    nc.sync.dma_start(rw1_t[:], rw1[0:2 * H, :])
                    rw1r = fin.tile([1, H], DT.float32)
                    nc.sync.dma_start(rw1r[:], rw1[2 * H:2 * H + 1, :])
                    rb1_t = fin.tile([H, 1], DT.float32)
                    nc.sync.dma_start(rb1_t[:], rb1[:, :])
                    rw2_t = fin.tile([H, 1], DT.float32)
                    nc.sync.dma_start(rw2_t[:], rw2[:, :])
                    rb2_t = fin.tile([1, 1], DT.float32)
                    nc.sync.dma_start(rb2_t[:], rb2c[:, :])
                    r1 = fps.tile([H, NG], DT.float32)
                    nc.tensor.matmul(r1[:], rw1_t[:], zt[:],
                                     start=True, stop=False,
                                     skip_group_check=True)
                    nc.tensor.matmul(r1[:], rw1r[:], dno[:],
                                     start=False, stop=True,
                                     skip_group_check=True)
                    r1s = fin.tile([H, NG], DT.float32)
                    nc.scalar.activation(r1s[:], r1[:],
                                         mybir.ActivationFunctionType.Relu,
                                         bias=rb1_t[:])
                    r2 = fps.tile([1, NG], DT.float32)
                    nc.tensor.matmul(r2[:], rw2_t[:], r1s[:], start=True,
                                     stop=True)
                    ot = fin.tile([1, NG], DT.float32)
                    nc.vector.tensor_scalar(ot[:], r2[:], rb2_t[:, 0:1],
                                            None, AluOpType.add)
                    nc.sync.dma_start(out[:, :], ot[:])

    nc.compile()
    return nc


# ---------------------------------------------------------------------------
# entry point
# ---------------------------------------------------------------------------

def kernel(**inputs):
    inp = {k: np.asarray(v) for k, v in inputs.items()}
    N = inp["state_x"].shape[0]
    NB = inp["depth"].shape[0]
    NG = NB // NCORES_ENC

    st = plan_encoder(inp["state_x"], inp["state_edge_index"],
                      inp["state_edge_type"], inp["state_batch"], NB)
    go = plan_encoder(inp["goal_x"], inp["goal_edge_index"],
                      inp["goal_edge_type"], inp["goal_batch"], NB)
    cores = st + go
    N_PAD = rup(max(c["n_own"] for c in cores), RANGE_D)
    NT1 = rup(N, CHUNK)
    nchunks1 = NT1 // CHUNK
    chunk2 = N_PAD // 2
    NT2 = NCORES_ENC * chunk2

    for c in cores:
        c["tsrc"] = c["src"]
        c["tpar"] = np.zeros(len(c["src"]), np.int64)
    p1_1 = build_phase1(cores, CHUNK, nchunks1, 1)
    p2_1 = build_phase2(p1_1, N_PAD)

    for enc, grp in ((0, st), (1, go)):
        nsb = np.array([c["ns"] for c in grp] + [N], np.int64)
        for c in grp:
            owner = np.searchsorted(nsb, c["src"], side="right") - 1
            loc = c["src"] - nsb[owner]
            c["tsrc"] = owner * chunk2 + loc // 2
            c["tpar"] = loc % 2
    p1_2 = build_phase1(cores, chunk2, NCORES_ENC, 2)
    p2_2 = build_phase2(p1_2, N_PAD)

    L1 = dict(p1=p1_1, p2=p2_1)
    L2 = dict(p1=p1_2, p2=p2_2)
    cfg = dict(N_PAD=N_PAD, NT1=NT1, NT2=NT2, L1=L1, L2=L2, NG=NG, NB=NB)
    nc = build_kernel(cfg)

    in_maps = []
    for ci in range(8):
        enc = ci // NCORES_ENC
        c = cores[ci]
        pfx = "s" if enc == 0 else "g"
        x = inp["state_x"] if enc == 0 else inp["goal_x"]
        batch = inp["state_batch"] if enc == 0 else inp["goal_batch"]
        xt = np.zeros((NT1, IN), BF16)
        xt[:N] = x.astype(BF16)
        xTv = np.zeros((IN, N_PAD), BF16)
        xTv[:, :c["n_own"]] = x[c["ns"]:c["ne"]].T.astype(BF16)
        W1 = np.ascontiguousarray(
            inp[pfx + "W1"].astype(BF16).transpose(1, 0, 2))
        W2s = np.zeros((IN, 2 * R, H), BF16)
        W2s[:H, :R] = inp[pfx + "W2"].astype(BF16).transpose(1, 0, 2)
        W2s[H:2 * H, R:] = inp[pfx + "W2"].astype(BF16).transpose(1, 0, 2)
        g0 = (ci % NCORES_ENC) * NG
        bown = batch[c["ns"]:c["ne"]] - g0
        gcnt = np.maximum(
            np.bincount(bown, minlength=NG).astype(np.float32), 1.0)
        pm = np.zeros((N_PAD // H, H, NG), BF16)
        nidx = np.arange(c["n_own"])
        pm[nidx // H, nidx % H, bown] = (1.0 / gcnt[bown]).astype(BF16)
        im = dict(
            xtab=xt, xT=xTv,
            w1=W1, wr1=inp[pfx + "Root1"].astype(BF16),
            w2=W2s, wr2=inp[pfx + "Root2"].astype(BF16),
            b1c=inp[pfx + "B1"].astype(np.float32)[:, None],
            b2c=inp[pfx + "B2"].astype(np.float32)[:, None],
            gi1=wrap16(p1_1["per_core"][ci]["gidx"]),
            gi2=wrap16(p1_2["per_core"][ci]["gidx"]),
            p2g1=wrap16(p2_1["per_core"][ci]["g2"]),
            p2g2=wrap16(p2_2["per_core"][ci]["g2"]),
            seg1=wrap128(p2_1["per_core"][ci]["seg"]),
            seg2=wrap128(p2_2["per_core"][ci]["seg"]),
            sc1=wrap128(p2_1["per_core"][ci]["sc2"]),
            sc2=wrap128(p2_2["per_core"][ci]["sc2"]),
            pmat=pm,
            depth=inp["depth"].astype(np.float32)[None, :],
            dep_own=inp["depth"].astype(np.float32)[None, g0:g0 + NG],
            rw1=inp["rW1"].astype(np.float32),
            rb1=inp["rB1"].astype(np.float32)[:, None],
            rw2=inp["rW2"].astype(np.float32),
            rb2c=inp["rB2"].astype(np.float32)[None, :],
            zsel=wrap16(np.r_[(ci % NCORES_ENC) * H + np.arange(H),
                              ((ci % NCORES_ENC) + NCORES_ENC) * H
                              + np.arange(H)].astype(np.int16)),
        )
        in_maps.append(im)

    res = bass_utils.run_bass_kernel_spmd(nc, in_maps,
                                          core_ids=list(range(8)))
    global LAST_DBG, LAST_PLANS, LAST_RES
    LAST_RES = res
    LAST_DBG = [res.results[c].get("dbg") for c in range(8)]
    LAST_PLANS = (cores, L1, L2, cfg)
    outp = np.zeros(NB, np.float32)
    for ci in range(NCORES_ENC):
        outp[ci * NG:(ci + 1) * NG] = res.results[ci]["out"][0]
    return outp

